# revision 1
# baseline (speedup 1.0000x reference)
"""Trainium2 Bass kernel for nn_ContinualSVGP (sparse-GP posterior prediction).

Math (per hyper h, output o; M=64 inducing, D=8, N=32768 points):
    kfu[n,m] = var * exp(-0.5*||x_n/ls - z_m/ls||^2)
    pred_mu  = kfu @ w            where w = Linv^T (Linv u_mean),  Linv = chol(kuu)^-1
    pred_var = var + diag(kfu (Q2-Q1) kfu^T),  Q1 = Kuu^-1, Q2 = C^T C,
               C = (u_tril / diag(L)) ^T Linv  (faithful to the reference's
               upper-triangular-solve-of-a-lower-matrix quirk).

Device mapping (per core, N sharded 8 ways -> N_loc=4096, blk=1024):
    mm1 (bf16 3-term split, K=102, ho-pair block-diag): s = W_aug^T xaug
    exp (ACT -> f32r):  kfu = exp(s)                      [128=2ho x 1024]
    mm2 (f32r, 2 chunks): t = blockdiag(Q,Q') kfu         [128 x 1024]
    prod (DVE -> bf16):   g = kfu * t
    mm3a (bf16, M=32, 4-window tile_position packing):
        psA rows 32w+{0..3} += ones . g   (pred_var - var), window w = pairs 2w,2w+1
    mm3b (f32r, (0,0), 2 chunks): psB rows 2p+s += w . kfu  (pred_mu)
    mmv (bf16 K=2) pre-writes psA with the var constants (var_hi+var_lo)
    DVE copies psA -> staging_var; ACT copies psB -> staging_mu; 2 DMAs out.
"""

import numpy as np
import ml_dtypes

H, O, M, D = 4, 4, 64, 8
N = 32768
JITTER = 1e-4
NCORES = 8
N_LOC = N // NCORES
BLK = 1024
NBLK = N_LOC // BLK
NHO = H * O          # 16
NPAIR = NHO // 2     # 8
KSPLIT = 3 * (D + D + 1)   # 51 rows per ho after 3-term bf16 split
BF16 = ml_dtypes.bfloat16

_cache = {}


def _rne11(a):
    """Round float32 array to f32r precision (RNE to 11 mantissa bits)."""
    b = np.asarray(a, np.float32).view(np.uint32)
    shift = 23 - 11
    add = np.uint32((1 << (shift - 1)) - 1)
    r = (((b + add + ((b >> np.uint32(shift)) & np.uint32(1))) >> np.uint32(shift))
         << np.uint32(shift))
    return r.view(np.float32)


def _bf16_split(v):
    """v (f64) -> (hi, lo) bf16 pair with hi+lo ~ v to ~2^-17."""
    hi = np.asarray(v, np.float64).astype(BF16)
    lo = (np.asarray(v, np.float64) - hi.astype(np.float64)).astype(BF16)
    return hi, lo


def _fwd_sub_inv(L):
    """Inverse of a lower-triangular matrix via forward substitution (f64)."""
    m = L.shape[0]
    inv = np.zeros_like(L)
    for i in range(m):
        inv[i, i] = 1.0 / L[i, i]
        for j in range(i):
            inv[i, j] = -np.dot(L[i, j:i], inv[j:i, j]) / L[i, i]
    return inv


def _host_precompute(x, z, u_mean, u_tril_vec, log_ls, log_var):
    """Build all device constants. Everything f64 internally."""
    x = x.astype(np.float64)
    z = z.astype(np.float64)
    um = u_mean.astype(np.float64)
    utv = u_tril_vec.astype(np.float64)
    lls = log_ls.astype(np.float64)
    lv = log_var.astype(np.float64)

    xr = np.empty((2 * D + 1, N), np.float64)
    xr[0:D] = x.T
    xr[D:2 * D] = (x.T) ** 2
    xr[2 * D] = 1.0
    x_hi, x_lo = _bf16_split(xr)
    xaug = np.empty((2 * KSPLIT, N), BF16)
    xaug[0:17] = x_hi
    xaug[17:34] = x_hi
    xaug[34:51] = x_lo
    xaug[51:102] = xaug[0:51]

    tril_i, tril_j = np.tril_indices(M)
    mm1w = np.zeros((2 * KSPLIT, NPAIR * 128), BF16)
    mm2w = np.zeros((128, NPAIR * 128), np.float32)
    mm3bw = np.zeros((128, NPAIR * 32), np.float32)
    mm3aw = np.zeros((128, NPAIR * 32), BF16)  # per pair: [128, 32]
    mmvw = np.zeros((2, 128), BF16)            # psA var pattern (K=2 split)

    for ho in range(NHO):
        h, o = divmod(ho, O)
        p, s = divmod(ho, 2)
        w_idx = p // 2          # window for mm3a
        ls = np.exp(lls[h, o])
        var = np.exp(lv[h, o])
        il2 = ls ** -2
        zs = z[o] / ls
        zn = (zs ** 2).sum(1)
        kuu = var * np.exp(-0.5 * (zn[:, None] + zn[None, :] - 2.0 * zs @ zs.T)) \
            + JITTER * np.eye(M)
        L = np.linalg.cholesky(kuu)
        Linv = _fwd_sub_inv(L)
        ut = np.zeros((M, M))
        ut[tril_i, tril_j] = utv[o]
        C = (ut / np.diag(L)[:, None]).T @ Linv
        Q = C.T @ C - Linv.T @ Linv
        w = Linv.T @ (Linv @ um[o][:, 0])

        ra = np.empty((2 * D + 1, M), np.float64)
        ra[0:D] = (z[o] * il2[None, :]).T
        ra[D:2 * D] = np.repeat((-0.5 * il2)[:, None], M, axis=1)
        ra[2 * D] = lv[h, o] - 0.5 * zn
        w_hi, w_lo = _bf16_split(ra)
        col0 = 64 * s
        mm1w[51 * s:51 * s + 17, 128 * p + col0:128 * p + col0 + 64] = w_hi
        mm1w[51 * s + 17:51 * s + 34, 128 * p + col0:128 * p + col0 + 64] = w_lo
        mm1w[51 * s + 34:51 * s + 51, 128 * p + col0:128 * p + col0 + 64] = w_hi

        mm2w[64 * s:64 * s + 64, 128 * p + col0:128 * p + col0 + 64] = \
            Q.astype(np.float32)
        # mm3a: per-pair block, local col 2*(p - 2*w_idx) + s
        mm3aw[64 * s:64 * s + 64, 32 * p + 2 * (p - 2 * w_idx) + s] = 1.0
        # mm3b: psA row 4 + 2p + s (window-0 bank, below the var rows)
        mm3bw[64 * s:64 * s + 64, 32 * p + 4 + 2 * p + s] = w.astype(np.float32)
        # mmv: psA row 32*w_idx + 2*(p-2*w_idx) + s
        row = 32 * w_idx + 2 * (p - 2 * w_idx) + s
        vh = np.float64(np.array(var, np.float64).astype(BF16))
        mmvw[0, row] = np.float32(vh)
        mmvw[1, row] = np.float32(var - vh)

    cR = np.concatenate([mm2w, mm3bw], axis=1).astype(BF16)  # [128, 1280]
    return xaug, mm1w, cR, mm3aw, mmvw


def _build_program():
    import concourse.bass as bass
    import concourse.mybir as mybir
    from concourse.tile import TileContext
    from concourse.tile_rust import add_dep_helper

    BF = mybir.dt.bfloat16
    FR = mybir.dt.float32r
    F32 = mybir.dt.float32

    nc = bass.Bass("TRN2", target_bir_lowering=False, debug=False,
                   num_devices=NCORES)
    xaug_ext = nc.dram_tensor("xaug", [2 * KSPLIT, N_LOC], BF,
                              kind="ExternalInput")
    mm1w_ext = nc.dram_tensor("mm1w", [2 * KSPLIT, NPAIR * 128], BF,
                              kind="ExternalInput")
    cr_ext = nc.dram_tensor("cR", [128, 1280], BF, kind="ExternalInput")
    m3a_ext = nc.dram_tensor("m3aw", [128, NPAIR * 32], BF,
                             kind="ExternalInput")
    mmvw_ext = nc.dram_tensor("mmvw", [2, 128], BF, kind="ExternalInput")
    ov_ext = nc.dram_tensor("outv", [128, N_LOC], F32, kind="ExternalOutput")

    with TileContext(nc) as tc:
        with tc.tile_pool(name="sb", bufs=1) as sb, \
             tc.tile_pool(name="kp", bufs=8) as kp, \
             tc.tile_pool(name="gp", bufs=8) as gp, \
             tc.tile_pool(name="st", bufs=3, space="PSUM") as stp, \
             tc.tile_pool(name="pa", bufs=1, space="PSUM") as pap:
            funnel = []
            xaug_d = sb.tile([2 * KSPLIT, N_LOC], BF, tag="xaug_d")
            funnel.append(nc.sync.dma_start(out=xaug_d[:], in_=xaug_ext[:]).ins)
            mm1w_d = sb.tile([2 * KSPLIT, NPAIR * 128], BF, tag="mm1w_d")
            funnel.append(nc.sync.dma_start(out=mm1w_d[:], in_=mm1w_ext[:]).ins)
            cr_d = sb.tile([128, 1280], BF, tag="cr_d")
            funnel.append(nc.sync.dma_start(out=cr_d[:], in_=cr_ext[:]).ins)
            m3a_d = sb.tile([128, NPAIR * 32], BF, tag="m3a_d")
            funnel.append(nc.sync.dma_start(out=m3a_d[:], in_=m3a_ext[:]).ins)
            mmvw_d = sb.tile([2, 128], BF, tag="mmvw_d")
            funnel.append(nc.sync.dma_start(out=mmvw_d[:], in_=mmvw_ext[:]).ins)

            # launder DMA'd inputs (DMA-queue waits never elide; engine sems do)
            xaug = sb.tile([2 * KSPLIT, N_LOC], BF, tag="xaug")
            nc.scalar.copy(xaug[:], xaug_d[:])
            mm1w = sb.tile([2 * KSPLIT, NPAIR * 128], BF, tag="mm1w")
            nc.scalar.copy(mm1w[:], mm1w_d[:])
            cr = sb.tile([128, 1280], BF, tag="cr")
            nc.vector.tensor_copy(cr[:], cr_d[:])
            m3aw = sb.tile([128, NPAIR * 32], BF, tag="m3aw")
            nc.vector.tensor_copy(m3aw[:], m3a_d[:])
            mmvw = sb.tile([2, 128], BF, tag="mmvw")
            nc.vector.tensor_copy(mmvw[:], mmvw_d[:])
            onesrow = sb.tile([2, BLK], BF, tag="onesrow")
            nc.vector.memset(onesrow[:], 1.0)
            dummy_bf = sb.tile([1, 1], BF, tag="dummy_bf")
            nc.vector.memset(dummy_bf[:], 0.0)
            dummy_srcA = sb.tile([1, 1], mybir.dt.float32, tag="dummy_srcA")
            nc.scalar.copy(dummy_srcA[:], dummy_bf[:])

            stag_v = sb.tile([128, N_LOC], mybir.dt.float32, tag="stag_v")

            prod_hist = []
            exp_hist = []
            mm2_hist = []
            last_pe = None
            last_dve_st = None
            last_act_st = None

            scv_prev = None
            for b in range(NBLK):
                psA = pap.tile([128, BLK], mybir.dt.float32, tag="psA")
                if scv_prev is not None:
                    ldwv = nc.tensor.ldweights(dummy_bf[:])
                    add_dep_helper(ldwv.ins, scv_prev, True,
                                   "PE observes stag_v copy before psA reuse")
                for c in range(2):
                    sl = slice(512 * c, 512 * (c + 1))
                    mmv = nc.tensor.matmul(psA[:, sl], mmvw[:],
                                           onesrow[:, sl],
                                           start=True, stop=False)
                    if scv_prev is not None:
                        add_dep_helper(mmv.ins, ldwv.ins, False, "order")
                blk_pre = []
                if b > 0:
                    prev_prod = prod_hist[b * NPAIR - 1]
                    prev_exp = exp_hist[b * NPAIR - 1]
                    t1 = sb.tile([1, 1], mybir.dt.float32, tag=f"aab1_{b}")
                    aab1 = nc.scalar.copy(t1[:], dummy_bf[:])
                    add_dep_helper(aab1.ins, prev_prod, True, "ACT sees DVE")
                    t2 = sb.tile([1, 1], mybir.dt.float32, tag=f"aab2_{b}")
                    aab2 = nc.scalar.copy(t2[:], dummy_srcA[:])
                    add_dep_helper(aab2.ins, prev_exp, True, "ACT WAW")
                    t3 = sb.tile([1, 1], mybir.dt.float32, tag=f"dvb_{b}")
                    dvb = nc.vector.memset(t3[:], 0.0)
                    add_dep_helper(dvb.ins, prev_prod, True, "DVE WAW")
                    blk_pre = [aab1.ins, aab2.ins, dvb.ins]

                for p in range(NPAIR):
                    it = b * NPAIR + p
                    w_idx = p // 2
                    ps_s = stp.tile([128, BLK], mybir.dt.float32, tag="st")
                    for c in range(2):
                        sl = slice(512 * c, 512 * (c + 1))
                        mm1 = nc.tensor.matmul(
                            ps_s[:, sl], mm1w[:, 128 * p:128 * (p + 1)],
                            xaug[:, BLK * b + 512 * c:BLK * b + 512 * (c + 1)],
                            start=True, stop=True)
                    kfu = kp.tile([128, BLK], BF, tag="kfu")
                    ex = nc.scalar.activation(
                        kfu[:], ps_s[:], mybir.ActivationFunctionType.Exp)
                    for pre in blk_pre:
                        add_dep_helper(ex.ins, pre, False, "after blk absorb")
                    exp_hist.append(ex.ins)
                    # absorb the ps_t slot's WAR (DVE prod of previous
                    # tenant) and PE WAW (mm1 wrote the slot this pair)
                    if it >= 1:
                        ldw = nc.tensor.ldweights(dummy_bf[:])
                        add_dep_helper(ldw.ins, prod_hist[it - 1], True,
                                       "absorb ps_t WAR")
                    ldw2 = nc.tensor.ldweights(dummy_bf[:])
                    add_dep_helper(ldw2.ins, ex.ins, True,
                                   "PE observes exp so mm2 keeps only WAW")
                    ps_t = stp.tile([128, BLK], mybir.dt.float32, tag="st")
                    mm2_first = None
                    for c in range(2):
                        sl = slice(512 * c, 512 * (c + 1))
                        mm2 = nc.tensor.matmul(ps_t[:, sl],
                                               cr[:, 128 * p:128 * (p + 1)],
                                               kfu[:, sl], start=True, stop=True)
                        if mm2_first is None:
                            mm2_first = mm2.ins
                            add_dep_helper(mm2.ins, ldw2.ins, False,
                                           "mm2 after WAW absorb")
                    mm2_hist.append(mm2.ins)
                    ddv = sb.tile([1, 1], mybir.dt.float32, tag=f"ddv{it}")
                    dab = nc.vector.memset(ddv[:], 0.0)
                    add_dep_helper(dab.ins, ex.ins, True, "absorb exp for DVE")
                    g = gp.tile([128, BLK], BF, tag="g")
                    pr = nc.vector.tensor_tensor(g[:], kfu[:], ps_t[:],
                                                 mybir.AluOpType.mult)
                    add_dep_helper(pr.ins, dab.ins, False, "order after absorb")
                    prod_hist.append(pr.ins)
                    # mm3a: bf16 window-packed var reduction
                    lc = 32 * p
                    for c in range(2):
                        sl = slice(512 * c, 512 * (c + 1))
                        nc.tensor.matmul(
                            psA[32 * w_idx:32 * w_idx + 32, sl],
                            m3aw[:, lc:lc + 32], g[:, sl],
                            start=False, stop=(p == NPAIR - 1),
                            tile_position=(0, 32 * w_idx))
                    # mm3b: f32r mu reduction at (0,0), 2 chunks
                    for c in range(2):
                        sl = slice(512 * c, 512 * (c + 1))
                        mm3b = nc.tensor.matmul(
                            psA[0:32, sl], cr[:, 1024 + 32 * p:1024 + 32 * (p + 1)],
                            kfu[:, sl], start=False, stop=False)
                        add_dep_helper(mm3b.ins, mm2_first, False,
                                       "mm3b after mm2 so ACT dep elides")
                    last_pe = mm3b.ins
                scv = nc.vector.tensor_copy(stag_v[:, BLK * b:BLK * (b + 1)],
                                            psA[:])
                scv_prev = scv.ins
                last_dve_st = scv.ins
                last_act_st = exp_hist[-1]

            dma_v = nc.sync.dma_start(out=ov_ext[:], in_=stag_v[:]).ins
            funnel += [dma_v, last_pe, last_dve_st, last_act_st,
                       prod_hist[-1]]
            for dep in funnel:
                nop = nc.sync.nop(nofuse=True)
                add_dep_helper(nop.ins, dep, True, "tail funnel")
    return nc


def kernel(x, z, u_mean, u_tril_vec, log_ls, log_var):
    from concourse.bass_utils import run_bass_kernel_spmd

    if "nc" not in _cache:
        _cache["nc"] = _build_program()
    nc = _cache["nc"]

    xaug, mm1w, cR, m3aw, mmvw = _host_precompute(
        np.asarray(x), np.asarray(z), np.asarray(u_mean),
        np.asarray(u_tril_vec), np.asarray(log_ls), np.asarray(log_var))

    in_maps = []
    for c in range(NCORES):
        in_maps.append({
            "xaug": np.ascontiguousarray(xaug[:, c * N_LOC:(c + 1) * N_LOC]),
            "mm1w": mm1w,
            "cR": cR.view(np.float32),
            "m3aw": m3aw,
            "mmvw": mmvw,
        })
    res = run_bass_kernel_spmd(nc, in_maps, list(range(NCORES)))
    outv = np.concatenate([res.results[c]["outv"] for c in range(NCORES)],
                          axis=1)             # [128, N]
    pred_var = np.empty((NHO, N), np.float32)
    pred_mu = np.empty((NHO, N), np.float32)
    for ho in range(NHO):
        p, s = divmod(ho, 2)
        w_idx = p // 2
        pred_var[ho] = outv[32 * w_idx + 2 * (p - 2 * w_idx) + s]
        pred_mu[ho] = outv[4 + 2 * p + s]
    return (pred_mu.reshape(H, O, N), pred_var.reshape(H, O, N))



# revision 19
# speedup vs baseline: 2.2301x; 2.2301x over previous
"""Trainium2 Bass kernel for nn_ContinualSVGP (sparse-GP posterior prediction).

Math (per hyper h, output o; M=64 inducing, D=8, N=32768 points):
    kfu[n,m] = var * exp(-0.5*||x_n/ls - z_m/ls||^2)
    pred_mu  = kfu @ w            where w = Linv^T (Linv u_mean),  Linv = chol(kuu)^-1
    pred_var = var + diag(kfu (Q2-Q1) kfu^T),  Q1 = Kuu^-1, Q2 = C^T C,
               C = (u_tril / diag(L)) ^T Linv  (faithful to the reference's
               upper-triangular-solve-of-a-lower-matrix quirk).

Device mapping (per core, N sharded 8 ways -> N_loc=4096, blk=1024):
    mm1 (bf16 3-term split, K=102, ho-pair block-diag): s = W_aug^T xaug
    exp (ACT -> f32r):  kfu = exp(s)                      [128=2ho x 1024]
    mm2 (f32r, 2 chunks): t = blockdiag(Q,Q') kfu         [128 x 1024]
    prod (DVE -> bf16):   g = kfu * t
    mm3a (bf16, M=32, 4-window tile_position packing):
        psA rows 32w+{0..3} += ones . g   (pred_var - var), window w = pairs 2w,2w+1
    mm3b (f32r, (0,0), 2 chunks): psB rows 2p+s += w . kfu  (pred_mu)
    mmv (bf16 K=2) pre-writes psA with the var constants (var_hi+var_lo)
    DVE copies psA -> staging; 4 tail DMAs emit only the 32 live rows.

Runner: the shard_map jit closure is built once and cached; the output
device buffer of call k is donated as the scratch output operand of call
k+1 (the program fully overwrites it), and output shards are fetched with
a thread pool.
"""

import numpy as np
import ml_dtypes
from concurrent.futures import ThreadPoolExecutor

H, O, M, D = 4, 4, 64, 8
N = 32768
JITTER = 1e-4
NCORES = 8
N_LOC = N // NCORES
BLK = 1024
NBLK = N_LOC // BLK
NHO = H * O          # 16
NPAIR = NHO // 2     # 8
KSPLIT = 3 * (D + D + 1)   # 51 rows per ho after 3-term bf16 split
BF16 = ml_dtypes.bfloat16
VAR_BASE = (0, 20, 24, 28)   # packed output row base per mm3a window

_cache = {}


def _bf16_split(v):
    """v (f64) -> (hi, lo) bf16 pair with hi+lo ~ v to ~2^-17."""
    hi = np.asarray(v, np.float64).astype(BF16)
    lo = (np.asarray(v, np.float64) - hi.astype(np.float64)).astype(BF16)
    return hi, lo


def _host_precompute(x, z, u_mean, u_tril_vec, log_ls, log_var):
    """Build all device constants. Everything f64 internally."""
    from scipy.linalg import solve_triangular

    x = x.astype(np.float64)
    z = z.astype(np.float64)
    um = u_mean.astype(np.float64)
    utv = u_tril_vec.astype(np.float64)
    lls = log_ls.astype(np.float64)
    lv = log_var.astype(np.float64)

    xr = np.empty((2 * D + 1, N), np.float64)
    xr[0:D] = x.T
    xr[D:2 * D] = (x.T) ** 2
    xr[2 * D] = 1.0
    x_hi, x_lo = _bf16_split(xr)
    xaug = np.empty((2 * KSPLIT, N), BF16)
    xaug[0:17] = x_hi
    xaug[17:34] = x_hi
    xaug[34:51] = x_lo
    xaug[51:102] = xaug[0:51]

    tril_i, tril_j = np.tril_indices(M)
    eye = np.eye(M)
    mm1w = np.zeros((2 * KSPLIT, NPAIR * 128), BF16)
    mm2w = np.zeros((128, NPAIR * 128), np.float32)
    mm3bw = np.zeros((128, NPAIR * 32), np.float32)
    mm3aw = np.zeros((128, NPAIR * 32), BF16)  # per pair: [128, 32]
    mmvw = np.zeros((2, 128), BF16)            # psA var pattern (K=2 split)

    for ho in range(NHO):
        h, o = divmod(ho, O)
        p, s = divmod(ho, 2)
        w_idx = p // 2          # window for mm3a
        ls = np.exp(lls[h, o])
        var = np.exp(lv[h, o])
        il2 = ls ** -2
        zs = z[o] / ls
        zn = (zs ** 2).sum(1)
        kuu = var * np.exp(-0.5 * (zn[:, None] + zn[None, :] - 2.0 * zs @ zs.T)) \
            + JITTER * eye
        L = np.linalg.cholesky(kuu)
        Linv = solve_triangular(L, eye, lower=True)
        ut = np.zeros((M, M))
        ut[tril_i, tril_j] = utv[o]
        C = (ut / np.diag(L)[:, None]).T @ Linv
        Q = C.T @ C - Linv.T @ Linv
        w = Linv.T @ (Linv @ um[o][:, 0])

        ra = np.empty((2 * D + 1, M), np.float64)
        ra[0:D] = (z[o] * il2[None, :]).T
        ra[D:2 * D] = np.repeat((-0.5 * il2)[:, None], M, axis=1)
        ra[2 * D] = lv[h, o] - 0.5 * zn
        w_hi, w_lo = _bf16_split(ra)
        col0 = 64 * s
        mm1w[51 * s:51 * s + 17, 128 * p + col0:128 * p + col0 + 64] = w_hi
        mm1w[51 * s + 17:51 * s + 34, 128 * p + col0:128 * p + col0 + 64] = w_lo
        mm1w[51 * s + 34:51 * s + 51, 128 * p + col0:128 * p + col0 + 64] = w_hi

        mm2w[64 * s:64 * s + 64, 128 * p + col0:128 * p + col0 + 64] = \
            Q.astype(np.float32)
        # mm3a: per-pair block, local col 2*(p - 2*w_idx) + s
        mm3aw[64 * s:64 * s + 64, 32 * p + 2 * (p - 2 * w_idx) + s] = 1.0
        # mm3b: psA row 4 + 2p + s (window-0 bank, below the var rows)
        mm3bw[64 * s:64 * s + 64, 32 * p + 4 + 2 * p + s] = w.astype(np.float32)
        # mmv: psA row 32*w_idx + 2*(p-2*w_idx) + s
        row = 32 * w_idx + 2 * (p - 2 * w_idx) + s
        vh = np.float64(np.array(var, np.float64).astype(BF16))
        mmvw[0, row] = np.float32(vh)
        mmvw[1, row] = np.float32(var - vh)

    cR = np.concatenate([mm2w, mm3bw], axis=1).astype(BF16)  # [128, 1280]
    return xaug, mm1w, cR, mm3aw, mmvw


def _build_program():
    import concourse.bass as bass
    import concourse.mybir as mybir
    from concourse.tile import TileContext
    from concourse.tile_rust import add_dep_helper

    BF = mybir.dt.bfloat16
    F32 = mybir.dt.float32

    nc = bass.Bass("TRN2", target_bir_lowering=False, debug=False,
                   num_devices=NCORES)
    xaug_ext = nc.dram_tensor("xaug", [2 * KSPLIT, N_LOC], BF,
                              kind="ExternalInput")
    mm1w_ext = nc.dram_tensor("mm1w", [2 * KSPLIT, NPAIR * 128], BF,
                              kind="ExternalInput")
    cr_ext = nc.dram_tensor("cR", [128, 1280], BF, kind="ExternalInput")
    m3a_ext = nc.dram_tensor("m3aw", [128, NPAIR * 32], BF,
                             kind="ExternalInput")
    mmvw_ext = nc.dram_tensor("mmvw", [2, 128], BF, kind="ExternalInput")
    ov_ext = nc.dram_tensor("outv", [32, N_LOC], F32, kind="ExternalOutput")

    with TileContext(nc) as tc:
        with tc.tile_pool(name="sb", bufs=1) as sb, \
             tc.tile_pool(name="kp", bufs=8) as kp, \
             tc.tile_pool(name="gp", bufs=8) as gp, \
             tc.tile_pool(name="st", bufs=3, space="PSUM") as stp, \
             tc.tile_pool(name="pa", bufs=1, space="PSUM") as pap:
            funnel = []
            xaug_d = sb.tile([2 * KSPLIT, N_LOC], BF, tag="xaug_d")
            funnel.append(nc.sync.dma_start(out=xaug_d[:], in_=xaug_ext[:]).ins)
            mm1w_d = sb.tile([2 * KSPLIT, NPAIR * 128], BF, tag="mm1w_d")
            funnel.append(nc.sync.dma_start(out=mm1w_d[:], in_=mm1w_ext[:]).ins)
            cr_d = sb.tile([128, 1280], BF, tag="cr_d")
            funnel.append(nc.sync.dma_start(out=cr_d[:], in_=cr_ext[:]).ins)
            m3a_d = sb.tile([128, NPAIR * 32], BF, tag="m3a_d")
            funnel.append(nc.sync.dma_start(out=m3a_d[:], in_=m3a_ext[:]).ins)
            mmvw_d = sb.tile([2, 128], BF, tag="mmvw_d")
            funnel.append(nc.sync.dma_start(out=mmvw_d[:], in_=mmvw_ext[:]).ins)

            # launder DMA'd inputs (DMA-queue waits never elide; engine sems do)
            xaug = sb.tile([2 * KSPLIT, N_LOC], BF, tag="xaug")
            nc.scalar.copy(xaug[:], xaug_d[:])
            mm1w = sb.tile([2 * KSPLIT, NPAIR * 128], BF, tag="mm1w")
            nc.scalar.copy(mm1w[:], mm1w_d[:])
            cr = sb.tile([128, 1280], BF, tag="cr")
            nc.vector.tensor_copy(cr[:], cr_d[:])
            m3aw = sb.tile([128, NPAIR * 32], BF, tag="m3aw")
            nc.vector.tensor_copy(m3aw[:], m3a_d[:])
            mmvw = sb.tile([2, 128], BF, tag="mmvw")
            nc.vector.tensor_copy(mmvw[:], mmvw_d[:])
            onesrow = sb.tile([2, BLK], BF, tag="onesrow")
            nc.vector.memset(onesrow[:], 1.0)
            dummy_bf = sb.tile([1, 1], BF, tag="dummy_bf")
            nc.vector.memset(dummy_bf[:], 0.0)
            dummy_srcA = sb.tile([1, 1], mybir.dt.float32, tag="dummy_srcA")
            nc.scalar.copy(dummy_srcA[:], dummy_bf[:])

            stag_v = sb.tile([128, N_LOC], mybir.dt.float32, tag="stag_v")

            prod_hist = []
            exp_hist = []
            mm2_hist = []
            last_pe = None
            last_dve_st = None
            last_act_st = None

            scv_prev = None
            for b in range(NBLK):
                psA = pap.tile([128, BLK], mybir.dt.float32, tag="psA")
                if scv_prev is not None:
                    ldwv = nc.tensor.ldweights(dummy_bf[:])
                    add_dep_helper(ldwv.ins, scv_prev, True,
                                   "PE observes stag_v copy before psA reuse")
                for c in range(2):
                    sl = slice(512 * c, 512 * (c + 1))
                    mmv = nc.tensor.matmul(psA[:, sl], mmvw[:],
                                           onesrow[:, sl],
                                           start=True, stop=False)
                    if scv_prev is not None:
                        add_dep_helper(mmv.ins, ldwv.ins, False, "order")
                blk_pre = []
                if b > 0:
                    prev_prod = prod_hist[b * NPAIR - 1]
                    prev_exp = exp_hist[b * NPAIR - 1]
                    t1 = sb.tile([1, 1], mybir.dt.float32, tag=f"aab1_{b}")
                    aab1 = nc.scalar.copy(t1[:], dummy_bf[:])
                    add_dep_helper(aab1.ins, prev_prod, True, "ACT sees DVE")
                    t2 = sb.tile([1, 1], mybir.dt.float32, tag=f"aab2_{b}")
                    aab2 = nc.scalar.copy(t2[:], dummy_srcA[:])
                    add_dep_helper(aab2.ins, prev_exp, True, "ACT WAW")
                    t3 = sb.tile([1, 1], mybir.dt.float32, tag=f"dvb_{b}")
                    dvb = nc.vector.memset(t3[:], 0.0)
                    add_dep_helper(dvb.ins, prev_prod, True, "DVE WAW")
                    blk_pre = [aab1.ins, aab2.ins, dvb.ins]

                for p in range(NPAIR):
                    it = b * NPAIR + p
                    w_idx = p // 2
                    ps_s = stp.tile([128, BLK], mybir.dt.float32, tag="st")
                    for c in range(2):
                        sl = slice(512 * c, 512 * (c + 1))
                        mm1 = nc.tensor.matmul(
                            ps_s[:, sl], mm1w[:, 128 * p:128 * (p + 1)],
                            xaug[:, BLK * b + 512 * c:BLK * b + 512 * (c + 1)],
                            start=True, stop=True)
                    kfu = kp.tile([128, BLK], BF, tag="kfu")
                    ex = nc.scalar.activation(
                        kfu[:], ps_s[:], mybir.ActivationFunctionType.Exp)
                    for pre in blk_pre:
                        add_dep_helper(ex.ins, pre, False, "after blk absorb")
                    exp_hist.append(ex.ins)
                    # absorb the ps_t slot's WAR (DVE prod of previous
                    # tenant) and PE WAW (mm1 wrote the slot this pair)
                    if it >= 1:
                        ldw = nc.tensor.ldweights(dummy_bf[:])
                        add_dep_helper(ldw.ins, prod_hist[it - 1], True,
                                       "absorb ps_t WAR")
                    ldw2 = nc.tensor.ldweights(dummy_bf[:])
                    add_dep_helper(ldw2.ins, ex.ins, True,
                                   "PE observes exp so mm2 keeps only WAW")
                    ps_t = stp.tile([128, BLK], mybir.dt.float32, tag="st")
                    mm2_first = None
                    for c in range(2):
                        sl = slice(512 * c, 512 * (c + 1))
                        mm2 = nc.tensor.matmul(ps_t[:, sl],
                                               cr[:, 128 * p:128 * (p + 1)],
                                               kfu[:, sl], start=True, stop=True)
                        if mm2_first is None:
                            mm2_first = mm2.ins
                            add_dep_helper(mm2.ins, ldw2.ins, False,
                                           "mm2 after WAW absorb")
                    mm2_hist.append(mm2.ins)
                    ddv = sb.tile([1, 1], mybir.dt.float32, tag=f"ddv{it}")
                    dab = nc.vector.memset(ddv[:], 0.0)
                    add_dep_helper(dab.ins, ex.ins, True, "absorb exp for DVE")
                    g = gp.tile([128, BLK], BF, tag="g")
                    pr = nc.vector.tensor_tensor(g[:], kfu[:], ps_t[:],
                                                 mybir.AluOpType.mult)
                    add_dep_helper(pr.ins, dab.ins, False, "order after absorb")
                    prod_hist.append(pr.ins)
                    # mm3a: bf16 window-packed var reduction
                    lc = 32 * p
                    for c in range(2):
                        sl = slice(512 * c, 512 * (c + 1))
                        nc.tensor.matmul(
                            psA[32 * w_idx:32 * w_idx + 32, sl],
                            m3aw[:, lc:lc + 32], g[:, sl],
                            start=False, stop=(p == NPAIR - 1),
                            tile_position=(0, 32 * w_idx))
                    # mm3b: f32r mu reduction at (0,0), 2 chunks
                    for c in range(2):
                        sl = slice(512 * c, 512 * (c + 1))
                        mm3b = nc.tensor.matmul(
                            psA[0:32, sl], cr[:, 1024 + 32 * p:1024 + 32 * (p + 1)],
                            kfu[:, sl], start=False, stop=False)
                        add_dep_helper(mm3b.ins, mm2_first, False,
                                       "mm3b after mm2 so ACT dep elides")
                    last_pe = mm3b.ins
                scv = nc.vector.tensor_copy(stag_v[:, BLK * b:BLK * (b + 1)],
                                            psA[:])
                scv_prev = scv.ins
                last_dve_st = scv.ins
                last_act_st = exp_hist[-1]

            # emit only the 32 live rows: var w0 + all mu, then var w1..w3.
            # 5 input DMAs keep the first tail DMA on a fresh semaphore
            # slot, so it carries only the staging-DVE wait (1-wait limit).
            funnel.append(nc.sync.dma_start(out=ov_ext[0:20, :],
                                            in_=stag_v[0:20, :]).ins)
            funnel.append(nc.sync.dma_start(out=ov_ext[20:24, :],
                                            in_=stag_v[32:36, :]).ins)
            funnel.append(nc.sync.dma_start(out=ov_ext[24:28, :],
                                            in_=stag_v[64:68, :]).ins)
            funnel.append(nc.sync.dma_start(out=ov_ext[28:32, :],
                                            in_=stag_v[96:100, :]).ins)
            funnel += [last_pe, last_dve_st, last_act_st, prod_hist[-1]]
            for dep in funnel:
                nop = nc.sync.nop(nofuse=True)
                add_dep_helper(nop.ins, dep, True, "tail funnel")
    return nc


def _build_runner():
    """Build the Bass program and a cached shard_map jit around bass_exec."""
    import jax
    from jax.sharding import Mesh, PartitionSpec
    from jax.experimental.shard_map import shard_map
    import concourse.mybir as mybir
    from concourse.bass2jax import (_bass_exec_p, partition_id_tensor,
                                    install_neuronx_cc_hook)

    nc = _build_program()
    install_neuronx_cc_hook()

    partition_name = (nc.partition_id_tensor.name
                      if nc.partition_id_tensor else None)
    in_names, out_names, out_avals = [], [], []
    for alloc in nc.m.functions[0].allocations:
        if not isinstance(alloc, mybir.MemoryLocationSet):
            continue
        name = alloc.memorylocations[0].name
        if alloc.kind == "ExternalInput":
            if name != partition_name:
                in_names.append(name)
        elif alloc.kind == "ExternalOutput":
            out_names.append(name)
            out_avals.append(jax.core.ShapedArray(
                tuple(alloc.tensor_shape), mybir.dt.np(alloc.dtype)))
    n_params = len(in_names)
    all_names = list(in_names) + list(out_names)
    if partition_name is not None:
        all_names.append(partition_name)

    def _body(*args):
        operands = list(args)
        if partition_name is not None:
            operands.append(partition_id_tensor())
        outs = _bass_exec_p.bind(
            *operands,
            out_avals=tuple(out_avals),
            in_names=tuple(all_names),
            out_names=tuple(out_names),
            lowering_input_output_aliases=(),
            sim_require_finite=True,
            sim_require_nnan=True,
            nc=nc,
        )
        return tuple(outs)

    devices = jax.devices()[:NCORES]
    mesh = Mesh(np.asarray(devices), ("core",))
    donate = tuple(range(n_params, n_params + len(out_names)))
    sharded = jax.jit(
        shard_map(_body, mesh=mesh,
                  in_specs=(PartitionSpec("core"),) * (n_params + len(out_names)),
                  out_specs=(PartitionSpec("core"),) * len(out_names),
                  check_rep=False),
        donate_argnums=donate, keep_unused=True)
    _cache["nc"] = nc
    _cache["sharded"] = sharded
    _cache["in_names"] = in_names
    _cache["donor"] = np.zeros((NCORES * 32, N_LOC), np.float32)
    _cache["pool"] = ThreadPoolExecutor(NCORES)


def kernel(x, z, u_mean, u_tril_vec, log_ls, log_var):
    if "sharded" not in _cache:
        _build_runner()

    xaug, mm1w, cR, m3aw, mmvw = _host_precompute(
        np.asarray(x), np.asarray(z), np.asarray(u_mean),
        np.asarray(u_tril_vec), np.asarray(log_ls), np.asarray(log_var))

    globals_by_name = {
        "xaug": xaug.reshape(2 * KSPLIT, NCORES, N_LOC).transpose(1, 0, 2)
                    .reshape(NCORES * 2 * KSPLIT, N_LOC),
        "mm1w": np.tile(mm1w, (NCORES, 1)),
        "cR": np.tile(cR, (NCORES, 1)),
        "m3aw": np.tile(m3aw, (NCORES, 1)),
        "mmvw": np.tile(mmvw, (NCORES, 1)),
    }
    args = [globals_by_name[n] for n in _cache["in_names"]]
    args.append(_cache["donor"])
    out = _cache["sharded"](*args)[0]
    _cache["donor"] = out

    shards = sorted(out.addressable_shards, key=lambda s: s.index[0].start)
    parts = list(_cache["pool"].map(lambda s: np.asarray(s.data), shards))
    full = np.concatenate(parts, axis=1)          # [32, N]

    pred_mu = np.empty((NHO, N), np.float32)
    pred_var = np.empty((NHO, N), np.float32)
    for ho in range(NHO):
        p, s = divmod(ho, 2)
        w_idx = p // 2
        pred_var[ho] = full[VAR_BASE[w_idx] + 2 * (p - 2 * w_idx) + s]
        pred_mu[ho] = full[4 + 2 * p + s]
    return (pred_mu.reshape(H, O, N), pred_var.reshape(H, O, N))


# revision 20
# speedup vs baseline: 3.7235x; 1.6696x over previous
"""Trainium2 Bass kernel for nn_ContinualSVGP (sparse-GP posterior prediction).

Math (per hyper h, output o; M=64 inducing, D=8, N=32768 points):
    kfu[n,m] = var * exp(-0.5*||x_n/ls - z_m/ls||^2)
    pred_mu  = kfu @ w            where w = Linv^T (Linv u_mean),  Linv = chol(kuu)^-1
    pred_var = var + diag(kfu (Q2-Q1) kfu^T),  Q1 = Kuu^-1, Q2 = C^T C,
               C = (u_tril / diag(L)) ^T Linv  (faithful to the reference's
               upper-triangular-solve-of-a-lower-matrix quirk).

Device mapping (per core, N sharded 8 ways -> N_loc=4096, blk=1024):
    mm1 (bf16 3-term split, K=102, ho-pair block-diag): s = W_aug^T xaug
    exp (ACT -> f32r):  kfu = exp(s)                      [128=2ho x 1024]
    mm2 (f32r, 2 chunks): t = blockdiag(Q,Q') kfu         [128 x 1024]
    prod (DVE -> bf16):   g = kfu * t
    mm3a (bf16, M=32, 4-window tile_position packing):
        psA rows 32w+{0..3} += ones . g   (pred_var - var), window w = pairs 2w,2w+1
    mm3b (f32r, (0,0), 2 chunks): psB rows 2p+s += w . kfu  (pred_mu)
    mmv (bf16 K=2) pre-writes psA with the var constants (var_hi+var_lo)
    DVE copies psA -> staging; 4 tail DMAs emit only the 32 live rows.

Runner: the shard_map jit closure is built once and cached; the output
device buffer of call k is donated as the scratch output operand of call
k+1 (the program fully overwrites it), and output shards are fetched with
a thread pool.
"""

import numpy as np
import ml_dtypes
from concurrent.futures import ThreadPoolExecutor

H, O, M, D = 4, 4, 64, 8
N = 32768
JITTER = 1e-4
NCORES = 8
N_LOC = N // NCORES
BLK = 1024
NBLK = N_LOC // BLK
NHO = H * O          # 16
NPAIR = NHO // 2     # 8
KSPLIT = 3 * (D + D + 1)   # 51 rows per ho after 3-term bf16 split
BF16 = ml_dtypes.bfloat16
VAR_BASE = (0, 20, 24, 28)   # packed output row base per mm3a window

_cache = {}


def _bf16_split(v):
    """v (f64) -> (hi, lo) bf16 pair with hi+lo ~ v to ~2^-17."""
    hi = np.asarray(v, np.float64).astype(BF16)
    lo = (np.asarray(v, np.float64) - hi.astype(np.float64)).astype(BF16)
    return hi, lo


def _host_precompute(x, z, u_mean, u_tril_vec, log_ls, log_var):
    """Build all device constants. Everything f64 internally."""
    from scipy.linalg import solve_triangular

    x = x.astype(np.float64)
    z = z.astype(np.float64)
    um = u_mean.astype(np.float64)
    utv = u_tril_vec.astype(np.float64)
    lls = log_ls.astype(np.float64)
    lv = log_var.astype(np.float64)

    xr = np.empty((2 * D + 1, N), np.float64)
    xr[0:D] = x.T
    xr[D:2 * D] = (x.T) ** 2
    xr[2 * D] = 1.0
    x_hi, x_lo = _bf16_split(xr)
    xaug = np.empty((2 * KSPLIT, N), BF16)
    xaug[0:17] = x_hi
    xaug[17:34] = x_hi
    xaug[34:51] = x_lo
    xaug[51:102] = xaug[0:51]

    tril_i, tril_j = np.tril_indices(M)
    eye = np.eye(M)
    mm1w = np.zeros((2 * KSPLIT, NPAIR * 128), BF16)
    mm2w = np.zeros((128, NPAIR * 128), np.float32)
    mm3bw = np.zeros((128, NPAIR * 32), np.float32)
    mm3aw = np.zeros((128, NPAIR * 32), BF16)  # per pair: [128, 32]
    mmvw = np.zeros((2, 128), BF16)            # psA var pattern (K=2 split)

    for ho in range(NHO):
        h, o = divmod(ho, O)
        p, s = divmod(ho, 2)
        w_idx = p // 2          # window for mm3a
        ls = np.exp(lls[h, o])
        var = np.exp(lv[h, o])
        il2 = ls ** -2
        zs = z[o] / ls
        zn = (zs ** 2).sum(1)
        kuu = var * np.exp(-0.5 * (zn[:, None] + zn[None, :] - 2.0 * zs @ zs.T)) \
            + JITTER * eye
        L = np.linalg.cholesky(kuu)
        Linv = solve_triangular(L, eye, lower=True)
        ut = np.zeros((M, M))
        ut[tril_i, tril_j] = utv[o]
        C = (ut / np.diag(L)[:, None]).T @ Linv
        Q = C.T @ C - Linv.T @ Linv
        w = Linv.T @ (Linv @ um[o][:, 0])

        ra = np.empty((2 * D + 1, M), np.float64)
        ra[0:D] = (z[o] * il2[None, :]).T
        ra[D:2 * D] = np.repeat((-0.5 * il2)[:, None], M, axis=1)
        ra[2 * D] = lv[h, o] - 0.5 * zn
        w_hi, w_lo = _bf16_split(ra)
        col0 = 64 * s
        mm1w[51 * s:51 * s + 17, 128 * p + col0:128 * p + col0 + 64] = w_hi
        mm1w[51 * s + 17:51 * s + 34, 128 * p + col0:128 * p + col0 + 64] = w_lo
        mm1w[51 * s + 34:51 * s + 51, 128 * p + col0:128 * p + col0 + 64] = w_hi

        mm2w[64 * s:64 * s + 64, 128 * p + col0:128 * p + col0 + 64] = \
            Q.astype(np.float32)
        # mm3a: per-pair block, local col 2*(p - 2*w_idx) + s
        mm3aw[64 * s:64 * s + 64, 32 * p + 2 * (p - 2 * w_idx) + s] = 1.0
        # mm3b: psA row 4 + 2p + s (window-0 bank, below the var rows)
        mm3bw[64 * s:64 * s + 64, 32 * p + 4 + 2 * p + s] = w.astype(np.float32)
        # mmv: psA row 32*w_idx + 2*(p-2*w_idx) + s
        row = 32 * w_idx + 2 * (p - 2 * w_idx) + s
        vh = np.float64(np.array(var, np.float64).astype(BF16))
        mmvw[0, row] = np.float32(vh)
        mmvw[1, row] = np.float32(var - vh)

    cR = np.concatenate([mm2w, mm3bw], axis=1).astype(BF16)  # [128, 1280]
    return xaug, mm1w, cR, mm3aw, mmvw


def _build_program():
    import concourse.bass as bass
    import concourse.mybir as mybir
    from concourse.tile import TileContext
    from concourse.tile_rust import add_dep_helper

    BF = mybir.dt.bfloat16
    F32 = mybir.dt.float32

    nc = bass.Bass("TRN2", target_bir_lowering=False, debug=False,
                   num_devices=NCORES)
    xaug_ext = nc.dram_tensor("xaug", [2 * KSPLIT, N_LOC], BF,
                              kind="ExternalInput")
    mm1w_ext = nc.dram_tensor("mm1w", [2 * KSPLIT, NPAIR * 128], BF,
                              kind="ExternalInput")
    cr_ext = nc.dram_tensor("cR", [128, 1280], BF, kind="ExternalInput")
    m3a_ext = nc.dram_tensor("m3aw", [128, NPAIR * 32], BF,
                             kind="ExternalInput")
    mmvw_ext = nc.dram_tensor("mmvw", [2, 128], BF, kind="ExternalInput")
    ov_ext = nc.dram_tensor("outv", [32, N_LOC], F32, kind="ExternalOutput")

    with TileContext(nc) as tc:
        with tc.tile_pool(name="sb", bufs=1) as sb, \
             tc.tile_pool(name="kp", bufs=8) as kp, \
             tc.tile_pool(name="gp", bufs=8) as gp, \
             tc.tile_pool(name="st", bufs=3, space="PSUM") as stp, \
             tc.tile_pool(name="pa", bufs=1, space="PSUM") as pap:
            funnel = []
            xaug_d = sb.tile([2 * KSPLIT, N_LOC], BF, tag="xaug_d")
            funnel.append(nc.sync.dma_start(out=xaug_d[:], in_=xaug_ext[:]).ins)
            mm1w_d = sb.tile([2 * KSPLIT, NPAIR * 128], BF, tag="mm1w_d")
            funnel.append(nc.sync.dma_start(out=mm1w_d[:], in_=mm1w_ext[:]).ins)
            cr_d = sb.tile([128, 1280], BF, tag="cr_d")
            funnel.append(nc.sync.dma_start(out=cr_d[:], in_=cr_ext[:]).ins)
            m3a_d = sb.tile([128, NPAIR * 32], BF, tag="m3a_d")
            funnel.append(nc.sync.dma_start(out=m3a_d[:], in_=m3a_ext[:]).ins)
            mmvw_d = sb.tile([2, 128], BF, tag="mmvw_d")
            funnel.append(nc.sync.dma_start(out=mmvw_d[:], in_=mmvw_ext[:]).ins)

            # launder DMA'd inputs (DMA-queue waits never elide; engine sems do)
            xaug = sb.tile([2 * KSPLIT, N_LOC], BF, tag="xaug")
            nc.scalar.copy(xaug[:], xaug_d[:])
            mm1w = sb.tile([2 * KSPLIT, NPAIR * 128], BF, tag="mm1w")
            nc.scalar.copy(mm1w[:], mm1w_d[:])
            cr = sb.tile([128, 1280], BF, tag="cr")
            nc.vector.tensor_copy(cr[:], cr_d[:])
            m3aw = sb.tile([128, NPAIR * 32], BF, tag="m3aw")
            nc.vector.tensor_copy(m3aw[:], m3a_d[:])
            mmvw = sb.tile([2, 128], BF, tag="mmvw")
            nc.vector.tensor_copy(mmvw[:], mmvw_d[:])
            onesrow = sb.tile([2, BLK], BF, tag="onesrow")
            nc.vector.memset(onesrow[:], 1.0)
            dummy_bf = sb.tile([1, 1], BF, tag="dummy_bf")
            nc.vector.memset(dummy_bf[:], 0.0)
            dummy_srcA = sb.tile([1, 1], mybir.dt.float32, tag="dummy_srcA")
            nc.scalar.copy(dummy_srcA[:], dummy_bf[:])

            stag_v = sb.tile([128, N_LOC], mybir.dt.float32, tag="stag_v")

            prod_hist = []
            exp_hist = []
            mm2_hist = []
            last_pe = None
            last_dve_st = None
            last_act_st = None

            scv_prev = None
            for b in range(NBLK):
                psA = pap.tile([128, BLK], mybir.dt.float32, tag="psA")
                if scv_prev is not None:
                    ldwv = nc.tensor.ldweights(dummy_bf[:])
                    add_dep_helper(ldwv.ins, scv_prev, True,
                                   "PE observes stag_v copy before psA reuse")
                for c in range(2):
                    sl = slice(512 * c, 512 * (c + 1))
                    mmv = nc.tensor.matmul(psA[:, sl], mmvw[:],
                                           onesrow[:, sl],
                                           start=True, stop=False)
                    if scv_prev is not None:
                        add_dep_helper(mmv.ins, ldwv.ins, False, "order")
                blk_pre = []
                if b > 0:
                    prev_prod = prod_hist[b * NPAIR - 1]
                    prev_exp = exp_hist[b * NPAIR - 1]
                    t1 = sb.tile([1, 1], mybir.dt.float32, tag=f"aab1_{b}")
                    aab1 = nc.scalar.copy(t1[:], dummy_bf[:])
                    add_dep_helper(aab1.ins, prev_prod, True, "ACT sees DVE")
                    t2 = sb.tile([1, 1], mybir.dt.float32, tag=f"aab2_{b}")
                    aab2 = nc.scalar.copy(t2[:], dummy_srcA[:])
                    add_dep_helper(aab2.ins, prev_exp, True, "ACT WAW")
                    t3 = sb.tile([1, 1], mybir.dt.float32, tag=f"dvb_{b}")
                    dvb = nc.vector.memset(t3[:], 0.0)
                    add_dep_helper(dvb.ins, prev_prod, True, "DVE WAW")
                    blk_pre = [aab1.ins, aab2.ins, dvb.ins]

                for p in range(NPAIR):
                    it = b * NPAIR + p
                    w_idx = p // 2
                    ps_s = stp.tile([128, BLK], mybir.dt.float32, tag="st")
                    for c in range(2):
                        sl = slice(512 * c, 512 * (c + 1))
                        mm1 = nc.tensor.matmul(
                            ps_s[:, sl], mm1w[:, 128 * p:128 * (p + 1)],
                            xaug[:, BLK * b + 512 * c:BLK * b + 512 * (c + 1)],
                            start=True, stop=True)
                    kfu = kp.tile([128, BLK], BF, tag="kfu")
                    ex = nc.scalar.activation(
                        kfu[:], ps_s[:], mybir.ActivationFunctionType.Exp)
                    for pre in blk_pre:
                        add_dep_helper(ex.ins, pre, False, "after blk absorb")
                    exp_hist.append(ex.ins)
                    # absorb the ps_t slot's WAR (DVE prod of previous
                    # tenant) and PE WAW (mm1 wrote the slot this pair)
                    if it >= 1:
                        ldw = nc.tensor.ldweights(dummy_bf[:])
                        add_dep_helper(ldw.ins, prod_hist[it - 1], True,
                                       "absorb ps_t WAR")
                    ldw2 = nc.tensor.ldweights(dummy_bf[:])
                    add_dep_helper(ldw2.ins, ex.ins, True,
                                   "PE observes exp so mm2 keeps only WAW")
                    ps_t = stp.tile([128, BLK], mybir.dt.float32, tag="st")
                    mm2_first = None
                    for c in range(2):
                        sl = slice(512 * c, 512 * (c + 1))
                        mm2 = nc.tensor.matmul(ps_t[:, sl],
                                               cr[:, 128 * p:128 * (p + 1)],
                                               kfu[:, sl], start=True, stop=True)
                        if mm2_first is None:
                            mm2_first = mm2.ins
                            add_dep_helper(mm2.ins, ldw2.ins, False,
                                           "mm2 after WAW absorb")
                    mm2_hist.append(mm2.ins)
                    ddv = sb.tile([1, 1], mybir.dt.float32, tag=f"ddv{it}")
                    dab = nc.vector.memset(ddv[:], 0.0)
                    add_dep_helper(dab.ins, ex.ins, True, "absorb exp for DVE")
                    g = gp.tile([128, BLK], BF, tag="g")
                    pr = nc.vector.tensor_tensor(g[:], kfu[:], ps_t[:],
                                                 mybir.AluOpType.mult)
                    add_dep_helper(pr.ins, dab.ins, False, "order after absorb")
                    prod_hist.append(pr.ins)
                    # mm3a: bf16 window-packed var reduction
                    lc = 32 * p
                    for c in range(2):
                        sl = slice(512 * c, 512 * (c + 1))
                        nc.tensor.matmul(
                            psA[32 * w_idx:32 * w_idx + 32, sl],
                            m3aw[:, lc:lc + 32], g[:, sl],
                            start=False, stop=(p == NPAIR - 1),
                            tile_position=(0, 32 * w_idx))
                    # mm3b: f32r mu reduction at (0,0), 2 chunks
                    for c in range(2):
                        sl = slice(512 * c, 512 * (c + 1))
                        mm3b = nc.tensor.matmul(
                            psA[0:32, sl], cr[:, 1024 + 32 * p:1024 + 32 * (p + 1)],
                            kfu[:, sl], start=False, stop=False)
                        add_dep_helper(mm3b.ins, mm2_first, False,
                                       "mm3b after mm2 so ACT dep elides")
                    last_pe = mm3b.ins
                scv = nc.vector.tensor_copy(stag_v[:, BLK * b:BLK * (b + 1)],
                                            psA[:])
                scv_prev = scv.ins
                last_dve_st = scv.ins
                last_act_st = exp_hist[-1]

            # emit only the 32 live rows: var w0 + all mu, then var w1..w3.
            # 5 input DMAs keep the first tail DMA on a fresh semaphore
            # slot, so it carries only the staging-DVE wait (1-wait limit).
            funnel.append(nc.sync.dma_start(out=ov_ext[0:20, :],
                                            in_=stag_v[0:20, :]).ins)
            funnel.append(nc.sync.dma_start(out=ov_ext[20:24, :],
                                            in_=stag_v[32:36, :]).ins)
            funnel.append(nc.sync.dma_start(out=ov_ext[24:28, :],
                                            in_=stag_v[64:68, :]).ins)
            funnel.append(nc.sync.dma_start(out=ov_ext[28:32, :],
                                            in_=stag_v[96:100, :]).ins)
            funnel += [last_pe, last_dve_st, last_act_st, prod_hist[-1]]
            for dep in funnel:
                nop = nc.sync.nop(nofuse=True)
                add_dep_helper(nop.ins, dep, True, "tail funnel")
    return nc


def _build_runner():
    """Build the Bass program and a cached shard_map jit around bass_exec."""
    import jax
    from jax.sharding import Mesh, PartitionSpec
    from jax.experimental.shard_map import shard_map
    import concourse.mybir as mybir
    from concourse.bass2jax import (_bass_exec_p, partition_id_tensor,
                                    install_neuronx_cc_hook)

    nc = _build_program()
    install_neuronx_cc_hook()

    partition_name = (nc.partition_id_tensor.name
                      if nc.partition_id_tensor else None)
    in_names, out_names, out_avals = [], [], []
    for alloc in nc.m.functions[0].allocations:
        if not isinstance(alloc, mybir.MemoryLocationSet):
            continue
        name = alloc.memorylocations[0].name
        if alloc.kind == "ExternalInput":
            if name != partition_name:
                in_names.append(name)
        elif alloc.kind == "ExternalOutput":
            out_names.append(name)
            out_avals.append(jax.core.ShapedArray(
                tuple(alloc.tensor_shape), mybir.dt.np(alloc.dtype)))
    n_params = len(in_names)
    all_names = list(in_names) + list(out_names)
    if partition_name is not None:
        all_names.append(partition_name)

    def _body(*args):
        operands = list(args)
        if partition_name is not None:
            operands.append(partition_id_tensor())
        outs = _bass_exec_p.bind(
            *operands,
            out_avals=tuple(out_avals),
            in_names=tuple(all_names),
            out_names=tuple(out_names),
            lowering_input_output_aliases=(),
            sim_require_finite=True,
            sim_require_nnan=True,
            nc=nc,
        )
        return tuple(outs)

    devices = jax.devices()[:NCORES]
    mesh = Mesh(np.asarray(devices), ("core",))
    donate = tuple(range(n_params, n_params + len(out_names)))
    sharded = jax.jit(
        shard_map(_body, mesh=mesh,
                  in_specs=(PartitionSpec("core"),) * (n_params + len(out_names)),
                  out_specs=(PartitionSpec("core"),) * len(out_names),
                  check_rep=False),
        donate_argnums=donate, keep_unused=True)
    _cache["nc"] = nc
    _cache["sharded"] = sharded
    _cache["in_names"] = in_names
    # device-resident donor so every call has the same arg signature
    # (numpy zeros on call 1 vs donated jax.Array later would retrace)
    from jax.sharding import NamedSharding
    _cache["donor"] = jax.device_put(
        np.zeros((NCORES * 32, N_LOC), np.float32),
        NamedSharding(mesh, PartitionSpec("core")))
    _cache["pool"] = ThreadPoolExecutor(NCORES)


def kernel(x, z, u_mean, u_tril_vec, log_ls, log_var):
    if "sharded" not in _cache:
        _build_runner()

    xaug, mm1w, cR, m3aw, mmvw = _host_precompute(
        np.asarray(x), np.asarray(z), np.asarray(u_mean),
        np.asarray(u_tril_vec), np.asarray(log_ls), np.asarray(log_var))

    globals_by_name = {
        "xaug": xaug.reshape(2 * KSPLIT, NCORES, N_LOC).transpose(1, 0, 2)
                    .reshape(NCORES * 2 * KSPLIT, N_LOC),
        "mm1w": np.tile(mm1w, (NCORES, 1)),
        "cR": np.tile(cR, (NCORES, 1)),
        "m3aw": np.tile(m3aw, (NCORES, 1)),
        "mmvw": np.tile(mmvw, (NCORES, 1)),
    }
    args = [globals_by_name[n] for n in _cache["in_names"]]
    args.append(_cache["donor"])
    out = _cache["sharded"](*args)[0]
    _cache["donor"] = out

    shards = sorted(out.addressable_shards, key=lambda s: s.index[0].start)
    parts = list(_cache["pool"].map(lambda s: np.asarray(s.data), shards))
    full = np.concatenate(parts, axis=1)          # [32, N]

    pred_mu = np.empty((NHO, N), np.float32)
    pred_var = np.empty((NHO, N), np.float32)
    for ho in range(NHO):
        p, s = divmod(ho, 2)
        w_idx = p // 2
        pred_var[ho] = full[VAR_BASE[w_idx] + 2 * (p - 2 * w_idx) + s]
        pred_mu[ho] = full[4 + 2 * p + s]
    return (pred_mu.reshape(H, O, N), pred_var.reshape(H, O, N))


# revision 25
# speedup vs baseline: 3.8402x; 1.0314x over previous
"""Trainium2 Bass kernel for nn_ContinualSVGP (sparse-GP posterior prediction).

Math (per hyper h, output o; M=64 inducing, D=8, N=32768 points):
    kfu[n,m] = var * exp(-0.5*||x_n/ls - z_m/ls||^2)
    pred_mu  = kfu @ w            where w = Linv^T (Linv u_mean),  Linv = chol(kuu)^-1
    pred_var = var + diag(kfu (Q2-Q1) kfu^T),  Q1 = Kuu^-1, Q2 = C^T C,
               C = (u_tril / diag(L)) ^T Linv  (faithful to the reference's
               upper-triangular-solve-of-a-lower-matrix quirk).

Device mapping (per core, N sharded 8 ways -> N_loc=4096, blk=1024):
    mm1 (bf16 3-term split, K=102, ho-pair block-diag): s = W_aug^T xaug
    exp (ACT -> f32r):  kfu = exp(s)                      [128=2ho x 1024]
    mm2 (f32r, 2 chunks): t = blockdiag(Q,Q') kfu         [128 x 1024]
    prod (DVE -> bf16):   g = kfu * t
    mm3a (bf16, M=32, 4-window tile_position packing):
        psA rows 32w+{0..3} += ones . g   (pred_var - var), window w = pairs 2w,2w+1
    mm3b (f32r, (0,0), 2 chunks): psB rows 2p+s += w . kfu  (pred_mu)
    mmv (bf16 K=2) pre-writes psA with the var constants (var_hi+var_lo)
    DVE copies psA -> staging; 4 tail DMAs emit only the 32 live rows.

Runner: the shard_map jit closure is built once and cached; the output
device buffer of call k is donated as the scratch output operand of call
k+1 (the program fully overwrites it), and output shards are fetched with
a thread pool.
"""

import numpy as np
import ml_dtypes
from concurrent.futures import ThreadPoolExecutor

H, O, M, D = 4, 4, 64, 8
N = 32768
JITTER = 1e-4
NCORES = 8
N_LOC = N // NCORES
BLK = 1024
NBLK = N_LOC // BLK
NHO = H * O          # 16
NPAIR = NHO // 2     # 8
KSPLIT = 3 * (D + D + 1)   # 51 rows per ho after 3-term bf16 split
BF16 = ml_dtypes.bfloat16
VAR_BASE = (0, 20, 24, 28)   # packed output row base per mm3a window

_cache = {}


def _bf16_split(v):
    """v (f64) -> (hi, lo) bf16 pair with hi+lo ~ v to ~2^-17."""
    hi = np.asarray(v, np.float64).astype(BF16)
    lo = (np.asarray(v, np.float64) - hi.astype(np.float64)).astype(BF16)
    return hi, lo


def _host_precompute(x, z, u_mean, u_tril_vec, log_ls, log_var):
    """Build all device constants. Everything f64 internally."""
    from scipy.linalg import solve_triangular

    x = x.astype(np.float64)
    z = z.astype(np.float64)
    um = u_mean.astype(np.float64)
    utv = u_tril_vec.astype(np.float64)
    lls = log_ls.astype(np.float64)
    lv = log_var.astype(np.float64)

    xr = np.empty((2 * D + 1, N), np.float64)
    xr[0:D] = x.T
    xr[D:2 * D] = (x.T) ** 2
    xr[2 * D] = 1.0
    x_hi, x_lo = _bf16_split(xr)
    xaug = np.empty((2 * KSPLIT, N), BF16)
    xaug[0:17] = x_hi
    xaug[17:34] = x_hi
    xaug[34:51] = x_lo
    xaug[51:102] = xaug[0:51]

    tril_i, tril_j = np.tril_indices(M)
    eye = np.eye(M)
    mm1w = np.zeros((2 * KSPLIT, NPAIR * 128), BF16)
    # pqs packs per-s-half Q blocks (cols 0:512), mu weights (512:520) and
    # the psA var pattern (rows 0:2, cols 520:648) — rows 64s:64s+64 hold
    # half s so every device-side expansion copy is partition-aligned
    pqs = np.zeros((128, 648), BF16)

    for ho in range(NHO):
        h, o = divmod(ho, O)
        p, s = divmod(ho, 2)
        w_idx = p // 2          # window for mm3a
        ls = np.exp(lls[h, o])
        var = np.exp(lv[h, o])
        il2 = ls ** -2
        zs = z[o] / ls
        zn = (zs ** 2).sum(1)
        kuu = var * np.exp(-0.5 * (zn[:, None] + zn[None, :] - 2.0 * zs @ zs.T)) \
            + JITTER * eye
        L = np.linalg.cholesky(kuu)
        Linv = solve_triangular(L, eye, lower=True)
        ut = np.zeros((M, M))
        ut[tril_i, tril_j] = utv[o]
        C = (ut / np.diag(L)[:, None]).T @ Linv
        Q = C.T @ C - Linv.T @ Linv
        w = Linv.T @ (Linv @ um[o][:, 0])

        ra = np.empty((2 * D + 1, M), np.float64)
        ra[0:D] = (z[o] * il2[None, :]).T
        ra[D:2 * D] = np.repeat((-0.5 * il2)[:, None], M, axis=1)
        ra[2 * D] = lv[h, o] - 0.5 * zn
        w_hi, w_lo = _bf16_split(ra)
        col0 = 64 * s
        mm1w[51 * s:51 * s + 17, 128 * p + col0:128 * p + col0 + 64] = w_hi
        mm1w[51 * s + 17:51 * s + 34, 128 * p + col0:128 * p + col0 + 64] = w_lo
        mm1w[51 * s + 34:51 * s + 51, 128 * p + col0:128 * p + col0 + 64] = w_hi

        pqs[64 * s:64 * s + 64, 64 * p:64 * p + 64] = \
            Q.astype(np.float32).astype(BF16)
        pqs[64 * s:64 * s + 64, 512 + p] = w.astype(np.float32).astype(BF16)
        # mmv: psA row 32*w_idx + 2*(p-2*w_idx) + s
        row = 32 * w_idx + 2 * (p - 2 * w_idx) + s
        vh = np.float64(np.array(var, np.float64).astype(BF16))
        pqs[0, 520 + row] = np.float32(vh)
        pqs[1, 520 + row] = np.float32(var - vh)

    return xaug, mm1w, pqs


def _build_program():
    import concourse.bass as bass
    import concourse.mybir as mybir
    from concourse.tile import TileContext
    from concourse.tile_rust import add_dep_helper

    BF = mybir.dt.bfloat16
    F32 = mybir.dt.float32

    nc = bass.Bass("TRN2", target_bir_lowering=False, debug=False,
                   num_devices=NCORES)
    xaug_ext = nc.dram_tensor("xaug", [2 * KSPLIT, N_LOC], BF,
                              kind="ExternalInput")
    mm1w_ext = nc.dram_tensor("mm1w", [2 * KSPLIT, NPAIR * 128], BF,
                              kind="ExternalInput")
    pqs_ext = nc.dram_tensor("pqs", [128, 648], BF, kind="ExternalInput")
    ov_ext = nc.dram_tensor("outv", [32, N_LOC], F32, kind="ExternalOutput")

    with TileContext(nc) as tc:
        with tc.tile_pool(name="sb", bufs=1) as sb, \
             tc.tile_pool(name="kp", bufs=8) as kp, \
             tc.tile_pool(name="gp", bufs=8) as gp, \
             tc.tile_pool(name="st", bufs=3, space="PSUM") as stp, \
             tc.tile_pool(name="pa", bufs=1, space="PSUM") as pap:
            funnel = []
            xaug_d = sb.tile([2 * KSPLIT, N_LOC], BF, tag="xaug_d")
            funnel.append(nc.sync.dma_start(out=xaug_d[:], in_=xaug_ext[:]).ins)
            mm1w_d = sb.tile([2 * KSPLIT, NPAIR * 128], BF, tag="mm1w_d")
            funnel.append(nc.sync.dma_start(out=mm1w_d[:], in_=mm1w_ext[:]).ins)
            pqs_d = sb.tile([128, 648], BF, tag="pqs_d")
            funnel.append(nc.sync.dma_start(out=pqs_d[:], in_=pqs_ext[:]).ins)

            # launder DMA'd inputs (DMA-queue waits never elide; engine sems do)
            xaug = sb.tile([2 * KSPLIT, N_LOC], BF, tag="xaug")
            nc.scalar.copy(xaug[:], xaug_d[:])
            mm1w = sb.tile([2 * KSPLIT, NPAIR * 128], BF, tag="mm1w")
            nc.scalar.copy(mm1w[:], mm1w_d[:])
            # cr built on device from the packed pqs: Q blocks + mu-weight
            # columns land at their block-diagonal positions (aligned copies)
            cr = sb.tile([128, 1280], BF, tag="cr")
            nc.vector.memset(cr[:], 0.0)
            for ho in range(NHO):
                p, s = divmod(ho, 2)
                c0 = 128 * p + 64 * s
                nc.vector.tensor_copy(
                    cr[64 * s:64 * s + 64, c0:c0 + 64],
                    pqs_d[64 * s:64 * s + 64, 64 * p:64 * p + 64])
                mc = 1024 + 32 * p + 4 + 2 * p + s
                nc.vector.tensor_copy(
                    cr[64 * s:64 * s + 64, mc:mc + 1],
                    pqs_d[64 * s:64 * s + 64, 512 + p:513 + p])
            # mm3a one-hot pattern built on device (input-independent)
            m3aw = sb.tile([128, NPAIR * 32], BF, tag="m3aw")
            nc.vector.memset(m3aw[:], 0.0)
            for ho in range(NHO):
                p, s = divmod(ho, 2)
                w_idx = p // 2
                mc = 32 * p + 2 * (p - 2 * w_idx) + s
                nc.vector.memset(m3aw[64 * s:64 * s + 64, mc:mc + 1], 1.0)
            mmvw = sb.tile([2, 128], BF, tag="mmvw")
            nc.vector.tensor_copy(mmvw[:], pqs_d[0:2, 520:648])
            onesrow = sb.tile([2, BLK], BF, tag="onesrow")
            nc.vector.memset(onesrow[:], 1.0)
            dummy_bf = sb.tile([1, 1], BF, tag="dummy_bf")
            nc.vector.memset(dummy_bf[:], 0.0)
            dummy_srcA = sb.tile([1, 1], mybir.dt.float32, tag="dummy_srcA")
            nc.scalar.copy(dummy_srcA[:], dummy_bf[:])

            stag_v = sb.tile([128, N_LOC], mybir.dt.float32, tag="stag_v")

            prod_hist = []
            exp_hist = []
            mm2_hist = []
            last_pe = None
            last_dve_st = None
            last_act_st = None

            scv_prev = None
            for b in range(NBLK):
                psA = pap.tile([128, BLK], mybir.dt.float32, tag="psA")
                if scv_prev is not None:
                    ldwv = nc.tensor.ldweights(dummy_bf[:])
                    add_dep_helper(ldwv.ins, scv_prev, True,
                                   "PE observes stag_v copy before psA reuse")
                for c in range(2):
                    sl = slice(512 * c, 512 * (c + 1))
                    mmv = nc.tensor.matmul(psA[:, sl], mmvw[:],
                                           onesrow[:, sl],
                                           start=True, stop=False)
                    if scv_prev is not None:
                        add_dep_helper(mmv.ins, ldwv.ins, False, "order")
                blk_pre = []
                if b > 0:
                    prev_prod = prod_hist[b * NPAIR - 1]
                    prev_exp = exp_hist[b * NPAIR - 1]
                    t1 = sb.tile([1, 1], mybir.dt.float32, tag=f"aab1_{b}")
                    aab1 = nc.scalar.copy(t1[:], dummy_bf[:])
                    add_dep_helper(aab1.ins, prev_prod, True, "ACT sees DVE")
                    t2 = sb.tile([1, 1], mybir.dt.float32, tag=f"aab2_{b}")
                    aab2 = nc.scalar.copy(t2[:], dummy_srcA[:])
                    add_dep_helper(aab2.ins, prev_exp, True, "ACT WAW")
                    t3 = sb.tile([1, 1], mybir.dt.float32, tag=f"dvb_{b}")
                    dvb = nc.vector.memset(t3[:], 0.0)
                    add_dep_helper(dvb.ins, prev_prod, True, "DVE WAW")
                    blk_pre = [aab1.ins, aab2.ins, dvb.ins]

                for p in range(NPAIR):
                    it = b * NPAIR + p
                    w_idx = p // 2
                    ps_s = stp.tile([128, BLK], mybir.dt.float32, tag="st")
                    for c in range(2):
                        sl = slice(512 * c, 512 * (c + 1))
                        mm1 = nc.tensor.matmul(
                            ps_s[:, sl], mm1w[:, 128 * p:128 * (p + 1)],
                            xaug[:, BLK * b + 512 * c:BLK * b + 512 * (c + 1)],
                            start=True, stop=True)
                    kfu = kp.tile([128, BLK], BF, tag="kfu")
                    ex = nc.scalar.activation(
                        kfu[:], ps_s[:], mybir.ActivationFunctionType.Exp)
                    for pre in blk_pre:
                        add_dep_helper(ex.ins, pre, False, "after blk absorb")
                    exp_hist.append(ex.ins)
                    # absorb the ps_t slot's WAR (DVE prod of previous
                    # tenant) and PE WAW (mm1 wrote the slot this pair)
                    if it >= 1:
                        ldw = nc.tensor.ldweights(dummy_bf[:])
                        add_dep_helper(ldw.ins, prod_hist[it - 1], True,
                                       "absorb ps_t WAR")
                    ldw2 = nc.tensor.ldweights(dummy_bf[:])
                    add_dep_helper(ldw2.ins, ex.ins, True,
                                   "PE observes exp so mm2 keeps only WAW")
                    ps_t = stp.tile([128, BLK], mybir.dt.float32, tag="st")
                    mm2_first = None
                    for c in range(2):
                        sl = slice(512 * c, 512 * (c + 1))
                        mm2 = nc.tensor.matmul(ps_t[:, sl],
                                               cr[:, 128 * p:128 * (p + 1)],
                                               kfu[:, sl], start=True, stop=True)
                        if mm2_first is None:
                            mm2_first = mm2.ins
                            add_dep_helper(mm2.ins, ldw2.ins, False,
                                           "mm2 after WAW absorb")
                    mm2_hist.append(mm2.ins)
                    ddv = sb.tile([1, 1], mybir.dt.float32, tag=f"ddv{it}")
                    dab = nc.vector.memset(ddv[:], 0.0)
                    add_dep_helper(dab.ins, ex.ins, True, "absorb exp for DVE")
                    g = gp.tile([128, BLK], BF, tag="g")
                    pr = nc.vector.tensor_tensor(g[:], kfu[:], ps_t[:],
                                                 mybir.AluOpType.mult)
                    add_dep_helper(pr.ins, dab.ins, False, "order after absorb")
                    prod_hist.append(pr.ins)
                    # mm3a: bf16 window-packed var reduction
                    lc = 32 * p
                    for c in range(2):
                        sl = slice(512 * c, 512 * (c + 1))
                        nc.tensor.matmul(
                            psA[32 * w_idx:32 * w_idx + 32, sl],
                            m3aw[:, lc:lc + 32], g[:, sl],
                            start=False, stop=(p == NPAIR - 1),
                            tile_position=(0, 32 * w_idx))
                    # mm3b: f32r mu reduction at (0,0), 2 chunks
                    for c in range(2):
                        sl = slice(512 * c, 512 * (c + 1))
                        mm3b = nc.tensor.matmul(
                            psA[0:32, sl], cr[:, 1024 + 32 * p:1024 + 32 * (p + 1)],
                            kfu[:, sl], start=False, stop=False)
                        add_dep_helper(mm3b.ins, mm2_first, False,
                                       "mm3b after mm2 so ACT dep elides")
                    last_pe = mm3b.ins
                scv = nc.vector.tensor_copy(stag_v[:, BLK * b:BLK * (b + 1)],
                                            psA[:])
                scv_prev = scv.ins
                last_dve_st = scv.ins
                last_act_st = exp_hist[-1]

            # emit only the 32 live rows: var w0 + all mu, then var w1..w3.
            # 5 input DMAs keep the first tail DMA on a fresh semaphore
            # slot, so it carries only the staging-DVE wait (1-wait limit).
            funnel.append(nc.sync.dma_start(out=ov_ext[0:20, :],
                                            in_=stag_v[0:20, :]).ins)
            funnel.append(nc.sync.dma_start(out=ov_ext[20:24, :],
                                            in_=stag_v[32:36, :]).ins)
            funnel.append(nc.sync.dma_start(out=ov_ext[24:28, :],
                                            in_=stag_v[64:68, :]).ins)
            funnel.append(nc.sync.dma_start(out=ov_ext[28:32, :],
                                            in_=stag_v[96:100, :]).ins)
            funnel += [last_pe, last_dve_st, last_act_st, prod_hist[-1]]
            for dep in funnel:
                nop = nc.sync.nop(nofuse=True)
                add_dep_helper(nop.ins, dep, True, "tail funnel")
    return nc


def _build_runner():
    """Build the Bass program and a cached shard_map jit around bass_exec."""
    import jax
    from jax.sharding import Mesh, PartitionSpec
    from jax.experimental.shard_map import shard_map
    import concourse.mybir as mybir
    from concourse.bass2jax import (_bass_exec_p, partition_id_tensor,
                                    install_neuronx_cc_hook)

    nc = _build_program()
    install_neuronx_cc_hook()

    partition_name = (nc.partition_id_tensor.name
                      if nc.partition_id_tensor else None)
    in_names, out_names, out_avals = [], [], []
    for alloc in nc.m.functions[0].allocations:
        if not isinstance(alloc, mybir.MemoryLocationSet):
            continue
        name = alloc.memorylocations[0].name
        if alloc.kind == "ExternalInput":
            if name != partition_name:
                in_names.append(name)
        elif alloc.kind == "ExternalOutput":
            out_names.append(name)
            out_avals.append(jax.core.ShapedArray(
                tuple(alloc.tensor_shape), mybir.dt.np(alloc.dtype)))
    n_params = len(in_names)
    all_names = list(in_names) + list(out_names)
    if partition_name is not None:
        all_names.append(partition_name)

    def _body(*args):
        operands = list(args)
        if partition_name is not None:
            operands.append(partition_id_tensor())
        outs = _bass_exec_p.bind(
            *operands,
            out_avals=tuple(out_avals),
            in_names=tuple(all_names),
            out_names=tuple(out_names),
            lowering_input_output_aliases=(),
            sim_require_finite=True,
            sim_require_nnan=True,
            nc=nc,
        )
        return tuple(outs)

    devices = jax.devices()[:NCORES]
    mesh = Mesh(np.asarray(devices), ("core",))
    donate = tuple(range(n_params, n_params + len(out_names)))
    sharded = jax.jit(
        shard_map(_body, mesh=mesh,
                  in_specs=(PartitionSpec("core"),) * (n_params + len(out_names)),
                  out_specs=(PartitionSpec("core"),) * len(out_names),
                  check_rep=False),
        donate_argnums=donate, keep_unused=True)
    _cache["nc"] = nc
    _cache["sharded"] = sharded
    _cache["in_names"] = in_names
    # device-resident donor so every call has the same arg signature
    # (numpy zeros on call 1 vs donated jax.Array later would retrace)
    from jax.sharding import NamedSharding
    _cache["donor"] = jax.device_put(
        np.zeros((NCORES * 32, N_LOC), np.float32),
        NamedSharding(mesh, PartitionSpec("core")))
    _cache["pool"] = ThreadPoolExecutor(NCORES)


def kernel(x, z, u_mean, u_tril_vec, log_ls, log_var):
    if "sharded" not in _cache:
        _build_runner()

    xaug, mm1w, pqs = _host_precompute(
        np.asarray(x), np.asarray(z), np.asarray(u_mean),
        np.asarray(u_tril_vec), np.asarray(log_ls), np.asarray(log_var))

    globals_by_name = {
        "xaug": xaug.reshape(2 * KSPLIT, NCORES, N_LOC).transpose(1, 0, 2)
                    .reshape(NCORES * 2 * KSPLIT, N_LOC),
        "mm1w": np.tile(mm1w, (NCORES, 1)),
        "pqs": np.tile(pqs, (NCORES, 1)),
    }
    args = [globals_by_name[n] for n in _cache["in_names"]]
    args.append(_cache["donor"])
    out = _cache["sharded"](*args)[0]
    _cache["donor"] = out

    shards = sorted(out.addressable_shards, key=lambda s: s.index[0].start)
    parts = list(_cache["pool"].map(lambda s: np.asarray(s.data), shards))
    full = np.concatenate(parts, axis=1)          # [32, N]

    pred_mu = np.empty((NHO, N), np.float32)
    pred_var = np.empty((NHO, N), np.float32)
    for ho in range(NHO):
        p, s = divmod(ho, 2)
        w_idx = p // 2
        pred_var[ho] = full[VAR_BASE[w_idx] + 2 * (p - 2 * w_idx) + s]
        pred_mu[ho] = full[4 + 2 * p + s]
    return (pred_mu.reshape(H, O, N), pred_var.reshape(H, O, N))


# revision 32
# speedup vs baseline: 4.7713x; 1.2425x over previous
"""Trainium2 Bass kernel for nn_ContinualSVGP (sparse-GP posterior prediction).

Math (per hyper h, output o; M=64 inducing, D=8, N=32768 points):
    kfu[n,m] = var * exp(-0.5*||x_n/ls - z_m/ls||^2)
    pred_mu  = kfu @ w            where w = Linv^T (Linv u_mean),  Linv = chol(kuu)^-1
    pred_var = var + diag(kfu (Q2-Q1) kfu^T),  Q1 = Kuu^-1, Q2 = C^T C,
               C = (u_tril / diag(L)) ^T Linv  (faithful to the reference's
               upper-triangular-solve-of-a-lower-matrix quirk).

Device mapping (per core, N sharded 8 ways -> N_loc=4096, blk=1024):
    mm1 (bf16 3-term split, K=102, ho-pair block-diag): s = W_aug^T xaug
    exp (ACT -> f32r):  kfu = exp(s)                      [128=2ho x 1024]
    mm2 (f32r, 2 chunks): t = blockdiag(Q,Q') kfu         [128 x 1024]
    prod (DVE -> bf16):   g = kfu * t
    mm3a (bf16, M=32, 4-window tile_position packing):
        psA rows 32w+{0..3} += ones . g   (pred_var - var), window w = pairs 2w,2w+1
    mm3b (f32r, (0,0), 2 chunks): psB rows 2p+s += w . kfu  (pred_mu)
    mmv (bf16 K=2) pre-writes psA with the var constants (var_hi+var_lo)
    DVE copies psA -> staging; 4 tail DMAs emit only the 32 live rows.

Runner: the shard_map jit closure is built once and cached; the output
device buffer of call k is donated as the scratch output operand of call
k+1 (the program fully overwrites it), and output shards are fetched with
a thread pool.
"""

import numpy as np
import ml_dtypes
from concurrent.futures import ThreadPoolExecutor

H, O, M, D = 4, 4, 64, 8
N = 32768
JITTER = 1e-4
NCORES = 8
N_LOC = N // NCORES
BLK = 1024
NBLK = N_LOC // BLK
NHO = H * O          # 16
NPAIR = NHO // 2     # 8
KSPLIT = 3 * (D + D + 1)   # 51 rows per ho after 3-term bf16 split
BF16 = ml_dtypes.bfloat16
VAR_BASE = (0, 20, 24, 28)   # packed output row base per mm3a window

_cache = {}


def _bf16_split(v):
    """v (f64) -> (hi, lo) bf16 pair with hi+lo ~ v to ~2^-17."""
    hi = np.asarray(v, np.float64).astype(BF16)
    lo = (np.asarray(v, np.float64) - hi.astype(np.float64)).astype(BF16)
    return hi, lo


def _host_precompute(x, z, u_mean, u_tril_vec, log_ls, log_var):
    """Build all device constants. Everything f64 internally."""
    from scipy.linalg import solve_triangular

    x = x.astype(np.float64)
    z = z.astype(np.float64)
    um = u_mean.astype(np.float64)
    utv = u_tril_vec.astype(np.float64)
    lls = log_ls.astype(np.float64)
    lv = log_var.astype(np.float64)

    xr = np.empty((2 * D + 1, N), np.float64)
    xr[0:D] = x.T
    xr[D:2 * D] = (x.T) ** 2
    xr[2 * D] = 1.0
    x_hi, x_lo = _bf16_split(xr)
    xs = np.empty((2 * (2 * D + 1), N), BF16)   # [34, N]: hi rows, lo rows
    xs[0:17] = x_hi
    xs[17:34] = x_lo

    tril_i, tril_j = np.tril_indices(M)
    eye = np.eye(M)
    # mm1 as two accumulating matmuls, all operands at partition base 0:
    #   K=34: [w_hi; w_hi] . [x_hi; x_lo]   (terms w_hi*x_hi + w_hi*x_lo)
    #   K=17: [w_lo] . [x_hi]               (term  w_lo*x_hi)
    w1 = np.zeros((2 * (2 * D + 1), NPAIR * 128), BF16)   # [34, 1024]
    w2 = np.zeros((2 * D + 1, NPAIR * 128), BF16)         # [17, 1024]
    # pqs packs per-s-half Q blocks (cols 0:512), mu weights (512:520) and
    # the psA var pattern (rows 0:2, cols 520:648) — rows 64s:64s+64 hold
    # half s so every device-side expansion copy is partition-aligned
    pqs = np.zeros((128, 648), BF16)

    for ho in range(NHO):
        h, o = divmod(ho, O)
        p, s = divmod(ho, 2)
        w_idx = p // 2          # window for mm3a
        ls = np.exp(lls[h, o])
        var = np.exp(lv[h, o])
        il2 = ls ** -2
        zs = z[o] / ls
        zn = (zs ** 2).sum(1)
        kuu = var * np.exp(-0.5 * (zn[:, None] + zn[None, :] - 2.0 * zs @ zs.T)) \
            + JITTER * eye
        L = np.linalg.cholesky(kuu)
        Linv = solve_triangular(L, eye, lower=True)
        ut = np.zeros((M, M))
        ut[tril_i, tril_j] = utv[o]
        C = (ut / np.diag(L)[:, None]).T @ Linv
        Q = C.T @ C - Linv.T @ Linv
        w = Linv.T @ (Linv @ um[o][:, 0])

        ra = np.empty((2 * D + 1, M), np.float64)
        ra[0:D] = (z[o] * il2[None, :]).T
        ra[D:2 * D] = np.repeat((-0.5 * il2)[:, None], M, axis=1)
        ra[2 * D] = lv[h, o] - 0.5 * zn
        w_hi, w_lo = _bf16_split(ra)
        c0 = 128 * p + 64 * s
        w1[0:17, c0:c0 + 64] = w_hi
        w1[17:34, c0:c0 + 64] = w_hi
        w2[:, c0:c0 + 64] = w_lo

        pqs[64 * s:64 * s + 64, 64 * p:64 * p + 64] = \
            Q.astype(np.float32).astype(BF16)
        pqs[64 * s:64 * s + 64, 512 + p] = w.astype(np.float32).astype(BF16)
        # mmv: psA row 32*w_idx + 2*(p-2*w_idx) + s
        row = 32 * w_idx + 2 * (p - 2 * w_idx) + s
        vh = np.float64(np.array(var, np.float64).astype(BF16))
        pqs[0, 520 + row] = np.float32(vh)
        pqs[1, 520 + row] = np.float32(var - vh)

    return xs, w1, w2, pqs


def _build_program():
    import concourse.bass as bass
    import concourse.mybir as mybir
    from concourse.tile import TileContext
    from concourse.tile_rust import add_dep_helper

    BF = mybir.dt.bfloat16
    F32 = mybir.dt.float32

    nc = bass.Bass("TRN2", target_bir_lowering=False, debug=False,
                   num_devices=NCORES)
    NF = 2 * D + 1    # 17 feature rows
    xs_ext = nc.dram_tensor("xs", [2 * NF, N_LOC], BF, kind="ExternalInput")
    w1_ext = nc.dram_tensor("w1", [2 * NF, NPAIR * 128], BF,
                            kind="ExternalInput")
    w2_ext = nc.dram_tensor("w2", [NF, NPAIR * 128], BF,
                            kind="ExternalInput")
    pqs_ext = nc.dram_tensor("pqs", [128, 648], BF, kind="ExternalInput")
    ov_ext = nc.dram_tensor("outv", [32, N_LOC], F32, kind="ExternalOutput")

    with TileContext(nc) as tc:
        with tc.tile_pool(name="sb", bufs=1) as sb, \
             tc.tile_pool(name="kp", bufs=8) as kp, \
             tc.tile_pool(name="gp", bufs=8) as gp, \
             tc.tile_pool(name="st", bufs=3, space="PSUM") as stp, \
             tc.tile_pool(name="pa", bufs=1, space="PSUM") as pap:
            funnel = []
            xs_d = sb.tile([2 * NF, N_LOC], BF, tag="xs_d")
            funnel.append(nc.sync.dma_start(out=xs_d[:], in_=xs_ext[:]).ins)
            w1_d = sb.tile([2 * NF, NPAIR * 128], BF, tag="w1_d")
            funnel.append(nc.sync.dma_start(out=w1_d[:], in_=w1_ext[:]).ins)
            w2_d = sb.tile([NF, NPAIR * 128], BF, tag="w2_d")
            funnel.append(nc.sync.dma_start(out=w2_d[:], in_=w2_ext[:]).ins)
            pqs_d = sb.tile([128, 648], BF, tag="pqs_d")
            funnel.append(nc.sync.dma_start(out=pqs_d[:], in_=pqs_ext[:]).ins)

            # launder DMA'd inputs (DMA-queue waits never elide; engine sems do)
            xs = sb.tile([2 * NF, N_LOC], BF, tag="xs")
            nc.scalar.copy(xs[:], xs_d[:])
            w1 = sb.tile([2 * NF, NPAIR * 128], BF, tag="w1")
            nc.scalar.copy(w1[:], w1_d[:])
            w2 = sb.tile([NF, NPAIR * 128], BF, tag="w2")
            nc.scalar.copy(w2[:], w2_d[:])
            # cr built on device from the packed pqs: Q blocks + mu-weight
            # columns land at their block-diagonal positions (aligned copies)
            cr = sb.tile([128, 1280], BF, tag="cr")
            nc.vector.memset(cr[:], 0.0)
            for ho in range(NHO):
                p, s = divmod(ho, 2)
                c0 = 128 * p + 64 * s
                nc.vector.tensor_copy(
                    cr[64 * s:64 * s + 64, c0:c0 + 64],
                    pqs_d[64 * s:64 * s + 64, 64 * p:64 * p + 64])
                mc = 1024 + 32 * p + 4 + 2 * p + s
                nc.vector.tensor_copy(
                    cr[64 * s:64 * s + 64, mc:mc + 1],
                    pqs_d[64 * s:64 * s + 64, 512 + p:513 + p])
            # mm3a one-hot pattern built on device (input-independent)
            m3aw = sb.tile([128, NPAIR * 32], BF, tag="m3aw")
            nc.vector.memset(m3aw[:], 0.0)
            for ho in range(NHO):
                p, s = divmod(ho, 2)
                w_idx = p // 2
                mc = 32 * p + 2 * (p - 2 * w_idx) + s
                nc.vector.memset(m3aw[64 * s:64 * s + 64, mc:mc + 1], 1.0)
            mmvw = sb.tile([2, 128], BF, tag="mmvw")
            nc.vector.tensor_copy(mmvw[:], pqs_d[0:2, 520:648])
            onesrow = sb.tile([2, BLK], BF, tag="onesrow")
            nc.vector.memset(onesrow[:], 1.0)
            dummy_bf = sb.tile([1, 1], BF, tag="dummy_bf")
            nc.vector.memset(dummy_bf[:], 0.0)
            dummy_srcA = sb.tile([1, 1], mybir.dt.float32, tag="dummy_srcA")
            nc.scalar.copy(dummy_srcA[:], dummy_bf[:])

            stag_v = sb.tile([128, N_LOC], mybir.dt.float32, tag="stag_v")

            prod_hist = []
            exp_hist = []
            mm2_hist = []
            last_pe = None
            last_dve_st = None
            last_act_st = None

            scv_prev = None
            for b in range(NBLK):
                psA = pap.tile([128, BLK], mybir.dt.float32, tag="psA")
                if scv_prev is not None:
                    ldwv = nc.tensor.ldweights(dummy_bf[:])
                    add_dep_helper(ldwv.ins, scv_prev, True,
                                   "PE observes stag_v copy before psA reuse")
                for c in range(2):
                    sl = slice(512 * c, 512 * (c + 1))
                    mmv = nc.tensor.matmul(psA[:, sl], mmvw[:],
                                           onesrow[:, sl],
                                           start=True, stop=False)
                    if scv_prev is not None:
                        add_dep_helper(mmv.ins, ldwv.ins, False, "order")
                blk_pre = []
                if b > 0:
                    prev_prod = prod_hist[b * NPAIR - 1]
                    prev_exp = exp_hist[b * NPAIR - 1]
                    t1 = sb.tile([1, 1], mybir.dt.float32, tag=f"aab1_{b}")
                    aab1 = nc.scalar.copy(t1[:], dummy_bf[:])
                    add_dep_helper(aab1.ins, prev_prod, True, "ACT sees DVE")
                    t2 = sb.tile([1, 1], mybir.dt.float32, tag=f"aab2_{b}")
                    aab2 = nc.scalar.copy(t2[:], dummy_srcA[:])
                    add_dep_helper(aab2.ins, prev_exp, True, "ACT WAW")
                    t3 = sb.tile([1, 1], mybir.dt.float32, tag=f"dvb_{b}")
                    dvb = nc.vector.memset(t3[:], 0.0)
                    add_dep_helper(dvb.ins, prev_prod, True, "DVE WAW")
                    blk_pre = [aab1.ins, aab2.ins, dvb.ins]

                for p in range(NPAIR):
                    it = b * NPAIR + p
                    w_idx = p // 2
                    ps_s = stp.tile([128, BLK], mybir.dt.float32, tag="st")
                    for c in range(2):
                        sl = slice(512 * c, 512 * (c + 1))
                        xsl = slice(BLK * b + 512 * c, BLK * b + 512 * (c + 1))
                        nc.tensor.matmul(
                            ps_s[:, sl], w1[:, 128 * p:128 * (p + 1)],
                            xs[:, xsl], start=True, stop=False)
                        nc.tensor.matmul(
                            ps_s[:, sl], w2[:, 128 * p:128 * (p + 1)],
                            xs[0:NF, xsl], start=False, stop=True)
                    kfu = kp.tile([128, BLK], BF, tag="kfu")
                    ex = nc.scalar.activation(
                        kfu[:], ps_s[:], mybir.ActivationFunctionType.Exp)
                    for pre in blk_pre:
                        add_dep_helper(ex.ins, pre, False, "after blk absorb")
                    exp_hist.append(ex.ins)
                    # absorb the ps_t slot's WAR (DVE prod of previous
                    # tenant) and PE WAW (mm1 wrote the slot this pair)
                    if it >= 1:
                        ldw = nc.tensor.ldweights(dummy_bf[:])
                        add_dep_helper(ldw.ins, prod_hist[it - 1], True,
                                       "absorb ps_t WAR")
                    ldw2 = nc.tensor.ldweights(dummy_bf[:])
                    add_dep_helper(ldw2.ins, ex.ins, True,
                                   "PE observes exp so mm2 keeps only WAW")
                    ps_t = stp.tile([128, BLK], mybir.dt.float32, tag="st")
                    mm2_first = None
                    for c in range(2):
                        sl = slice(512 * c, 512 * (c + 1))
                        mm2 = nc.tensor.matmul(ps_t[:, sl],
                                               cr[:, 128 * p:128 * (p + 1)],
                                               kfu[:, sl], start=True, stop=True)
                        if mm2_first is None:
                            mm2_first = mm2.ins
                            add_dep_helper(mm2.ins, ldw2.ins, False,
                                           "mm2 after WAW absorb")
                    mm2_hist.append(mm2.ins)
                    ddv = sb.tile([1, 1], mybir.dt.float32, tag=f"ddv{it}")
                    dab = nc.vector.memset(ddv[:], 0.0)
                    add_dep_helper(dab.ins, ex.ins, True, "absorb exp for DVE")
                    g = gp.tile([128, BLK], BF, tag="g")
                    pr = nc.vector.tensor_tensor(g[:], kfu[:], ps_t[:],
                                                 mybir.AluOpType.mult)
                    add_dep_helper(pr.ins, dab.ins, False, "order after absorb")
                    prod_hist.append(pr.ins)
                    # mm3a: bf16 window-packed var reduction
                    lc = 32 * p
                    for c in range(2):
                        sl = slice(512 * c, 512 * (c + 1))
                        nc.tensor.matmul(
                            psA[32 * w_idx:32 * w_idx + 32, sl],
                            m3aw[:, lc:lc + 32], g[:, sl],
                            start=False, stop=(p == NPAIR - 1),
                            tile_position=(0, 32 * w_idx))
                    # mm3b: f32r mu reduction at (0,0), 2 chunks
                    for c in range(2):
                        sl = slice(512 * c, 512 * (c + 1))
                        mm3b = nc.tensor.matmul(
                            psA[0:32, sl], cr[:, 1024 + 32 * p:1024 + 32 * (p + 1)],
                            kfu[:, sl], start=False, stop=False)
                        add_dep_helper(mm3b.ins, mm2_first, False,
                                       "mm3b after mm2 so ACT dep elides")
                    last_pe = mm3b.ins
                scv = nc.vector.tensor_copy(stag_v[:, BLK * b:BLK * (b + 1)],
                                            psA[:])
                scv_prev = scv.ins
                last_dve_st = scv.ins
                last_act_st = exp_hist[-1]

            # emit only the 32 live rows: var w0 + all mu, then var w1..w3.
            # 5 input DMAs keep the first tail DMA on a fresh semaphore
            # slot, so it carries only the staging-DVE wait (1-wait limit).
            funnel.append(nc.sync.dma_start(out=ov_ext[0:20, :],
                                            in_=stag_v[0:20, :]).ins)
            funnel.append(nc.sync.dma_start(out=ov_ext[20:24, :],
                                            in_=stag_v[32:36, :]).ins)
            funnel.append(nc.sync.dma_start(out=ov_ext[24:28, :],
                                            in_=stag_v[64:68, :]).ins)
            funnel.append(nc.sync.dma_start(out=ov_ext[28:32, :],
                                            in_=stag_v[96:100, :]).ins)
            funnel += [last_pe, last_dve_st, last_act_st, prod_hist[-1]]
            for dep in funnel:
                nop = nc.sync.nop(nofuse=True)
                add_dep_helper(nop.ins, dep, True, "tail funnel")
    return nc


def _build_runner():
    """Build the Bass program and a cached shard_map jit around bass_exec."""
    import jax
    from jax.sharding import Mesh, PartitionSpec
    from jax.experimental.shard_map import shard_map
    import concourse.mybir as mybir
    from concourse.bass2jax import (_bass_exec_p, partition_id_tensor,
                                    install_neuronx_cc_hook)

    nc = _build_program()
    install_neuronx_cc_hook()

    partition_name = (nc.partition_id_tensor.name
                      if nc.partition_id_tensor else None)
    in_names, out_names, out_avals = [], [], []
    for alloc in nc.m.functions[0].allocations:
        if not isinstance(alloc, mybir.MemoryLocationSet):
            continue
        name = alloc.memorylocations[0].name
        if alloc.kind == "ExternalInput":
            if name != partition_name:
                in_names.append(name)
        elif alloc.kind == "ExternalOutput":
            out_names.append(name)
            out_avals.append(jax.core.ShapedArray(
                tuple(alloc.tensor_shape), mybir.dt.np(alloc.dtype)))
    n_params = len(in_names)
    all_names = list(in_names) + list(out_names)
    if partition_name is not None:
        all_names.append(partition_name)

    def _body(*args):
        operands = list(args)
        if partition_name is not None:
            operands.append(partition_id_tensor())
        outs = _bass_exec_p.bind(
            *operands,
            out_avals=tuple(out_avals),
            in_names=tuple(all_names),
            out_names=tuple(out_names),
            lowering_input_output_aliases=(),
            sim_require_finite=True,
            sim_require_nnan=True,
            nc=nc,
        )
        return tuple(outs)

    devices = jax.devices()[:NCORES]
    mesh = Mesh(np.asarray(devices), ("core",))
    donate = tuple(range(n_params, n_params + len(out_names)))
    sharded = jax.jit(
        shard_map(_body, mesh=mesh,
                  in_specs=(PartitionSpec("core"),) * (n_params + len(out_names)),
                  out_specs=(PartitionSpec("core"),) * len(out_names),
                  check_rep=False),
        donate_argnums=donate, keep_unused=True)
    _cache["nc"] = nc
    _cache["sharded"] = sharded
    _cache["in_names"] = in_names
    # device-resident donor so every call has the same arg signature
    # (numpy zeros on call 1 vs donated jax.Array later would retrace)
    from jax.sharding import NamedSharding
    _cache["donor"] = jax.device_put(
        np.zeros((NCORES * 32, N_LOC), np.float32),
        NamedSharding(mesh, PartitionSpec("core")))
    _cache["pool"] = ThreadPoolExecutor(NCORES)


def kernel(x, z, u_mean, u_tril_vec, log_ls, log_var):
    if "sharded" not in _cache:
        _build_runner()

    xs, w1, w2, pqs = _host_precompute(
        np.asarray(x), np.asarray(z), np.asarray(u_mean),
        np.asarray(u_tril_vec), np.asarray(log_ls), np.asarray(log_var))

    globals_by_name = {
        "xs": xs.reshape(34, NCORES, N_LOC).transpose(1, 0, 2)
                .reshape(NCORES * 34, N_LOC),
        "w1": np.tile(w1, (NCORES, 1)),
        "w2": np.tile(w2, (NCORES, 1)),
        "pqs": np.tile(pqs, (NCORES, 1)),
    }
    args = [globals_by_name[n] for n in _cache["in_names"]]
    args.append(_cache["donor"])
    out = _cache["sharded"](*args)[0]
    _cache["donor"] = out

    shards = sorted(out.addressable_shards, key=lambda s: s.index[0].start)
    parts = list(_cache["pool"].map(lambda s: np.asarray(s.data), shards))
    full = np.concatenate(parts, axis=1)          # [32, N]

    pred_mu = np.empty((NHO, N), np.float32)
    pred_var = np.empty((NHO, N), np.float32)
    for ho in range(NHO):
        p, s = divmod(ho, 2)
        w_idx = p // 2
        pred_var[ho] = full[VAR_BASE[w_idx] + 2 * (p - 2 * w_idx) + s]
        pred_mu[ho] = full[4 + 2 * p + s]
    return (pred_mu.reshape(H, O, N), pred_var.reshape(H, O, N))


# revision 36
# speedup vs baseline: 5.9910x; 1.2556x over previous
"""Trainium2 Bass kernel for nn_ContinualSVGP (sparse-GP posterior prediction).

Math (per hyper h, output o; M=64 inducing, D=8, N=32768 points):
    kfu[n,m] = var * exp(-0.5*||x_n/ls - z_m/ls||^2)
    pred_mu  = kfu @ w            where w = Linv^T (Linv u_mean),  Linv = chol(kuu)^-1
    pred_var = var + diag(kfu (Q2-Q1) kfu^T),  Q1 = Kuu^-1, Q2 = C^T C,
               C = (u_tril / diag(L)) ^T Linv  (faithful to the reference's
               upper-triangular-solve-of-a-lower-matrix quirk).

Device mapping (per core, N sharded 8 ways -> N_loc=4096, blk=1024):
    mm1 (bf16 3-term split, K=102, ho-pair block-diag): s = W_aug^T xaug
    exp (ACT -> f32r):  kfu = exp(s)                      [128=2ho x 1024]
    mm2 (f32r, 2 chunks): t = blockdiag(Q,Q') kfu         [128 x 1024]
    prod (DVE -> bf16):   g = kfu * t
    mm3a (bf16, M=32, 4-window tile_position packing):
        psA rows 32w+{0..3} += ones . g   (pred_var - var), window w = pairs 2w,2w+1
    mm3b (f32r, (0,0), 2 chunks): psB rows 2p+s += w . kfu  (pred_mu)
    mmv (bf16 K=2) pre-writes psA with the var constants (var_hi+var_lo)
    DVE copies psA -> staging; 4 tail DMAs emit only the 32 live rows.

Runner: the shard_map jit closure is built once and cached; the output
device buffer of call k is donated as the scratch output operand of call
k+1 (the program fully overwrites it), and output shards are fetched with
a thread pool.
"""

import numpy as np
import ml_dtypes
from concurrent.futures import ThreadPoolExecutor

H, O, M, D = 4, 4, 64, 8
N = 32768
JITTER = 1e-4
NCORES = 8
N_LOC = N // NCORES
BLK = 1024
NBLK = N_LOC // BLK
NHO = H * O          # 16
NPAIR = NHO // 2     # 8
KSPLIT = 3 * (D + D + 1)   # 51 rows per ho after 3-term bf16 split
BF16 = ml_dtypes.bfloat16
VAR_BASE = (0, 20, 24, 28)   # packed output row base per mm3a window

_cache = {}


def _bf16_split(v):
    """v (f64) -> (hi, lo) bf16 pair with hi+lo ~ v to ~2^-17."""
    hi = np.asarray(v, np.float64).astype(BF16)
    lo = (np.asarray(v, np.float64) - hi.astype(np.float64)).astype(BF16)
    return hi, lo


def _host_precompute(x, z, u_mean, u_tril_vec, log_ls, log_var):
    """Build all device constants. Everything f64 internally."""
    from scipy.linalg import solve_triangular

    x = x.astype(np.float64)
    z = z.astype(np.float64)
    um = u_mean.astype(np.float64)
    utv = u_tril_vec.astype(np.float64)
    lls = log_ls.astype(np.float64)
    lv = log_var.astype(np.float64)

    xr = np.empty((2 * D + 1, N), np.float64)
    xr[0:D] = x.T
    xr[D:2 * D] = (x.T) ** 2
    xr[2 * D] = 1.0
    x_hi, x_lo = _bf16_split(xr)
    xs = np.empty((2 * (2 * D + 1), N), BF16)   # [34, N]: hi rows, lo rows
    xs[0:17] = x_hi
    xs[17:34] = x_lo

    tril_i, tril_j = np.tril_indices(M)
    eye = np.eye(M)
    # mm1 as two accumulating matmuls, all operands at partition base 0:
    #   K=34: [w_hi; w_hi] . [x_hi; x_lo]   (terms w_hi*x_hi + w_hi*x_lo)
    #   K=17: [w_lo] . [x_hi]               (term  w_lo*x_hi)
    w1 = np.zeros((2 * (2 * D + 1), NPAIR * 128), BF16)   # [34, 1024]
    w2 = np.zeros((2 * D + 1, NPAIR * 128), BF16)         # [17, 1024]
    # pqs packs per-s-half Q blocks (cols 0:512), mu weights (512:520) and
    # the psA var pattern (rows 0:2, cols 520:648) — rows 64s:64s+64 hold
    # half s so every device-side expansion copy is partition-aligned
    pqs = np.zeros((128, 648), BF16)

    for ho in range(NHO):
        h, o = divmod(ho, O)
        p, s = divmod(ho, 2)
        w_idx = p // 2          # window for mm3a
        ls = np.exp(lls[h, o])
        var = np.exp(lv[h, o])
        il2 = ls ** -2
        zs = z[o] / ls
        zn = (zs ** 2).sum(1)
        kuu = var * np.exp(-0.5 * (zn[:, None] + zn[None, :] - 2.0 * zs @ zs.T)) \
            + JITTER * eye
        L = np.linalg.cholesky(kuu)
        Linv = solve_triangular(L, eye, lower=True)
        ut = np.zeros((M, M))
        ut[tril_i, tril_j] = utv[o]
        C = (ut / np.diag(L)[:, None]).T @ Linv
        Q = C.T @ C - Linv.T @ Linv
        w = Linv.T @ (Linv @ um[o][:, 0])

        ra = np.empty((2 * D + 1, M), np.float64)
        ra[0:D] = (z[o] * il2[None, :]).T
        ra[D:2 * D] = np.repeat((-0.5 * il2)[:, None], M, axis=1)
        ra[2 * D] = lv[h, o] - 0.5 * zn
        w_hi, w_lo = _bf16_split(ra)
        c0 = 128 * p + 64 * s
        w1[0:17, c0:c0 + 64] = w_hi
        w1[17:34, c0:c0 + 64] = w_hi
        w2[:, c0:c0 + 64] = w_lo

        pqs[64 * s:64 * s + 64, 64 * p:64 * p + 64] = \
            Q.astype(np.float32).astype(BF16)
        pqs[64 * s:64 * s + 64, 512 + p] = w.astype(np.float32).astype(BF16)
        # mmv: psA row 32*w_idx + 2*(p-2*w_idx) + s
        row = 32 * w_idx + 2 * (p - 2 * w_idx) + s
        vh = np.float64(np.array(var, np.float64).astype(BF16))
        pqs[0, 520 + row] = np.float32(vh)
        pqs[1, 520 + row] = np.float32(var - vh)

    return xs, w1, w2, pqs


def _build_program():
    import concourse.bass as bass
    import concourse.mybir as mybir
    from concourse.tile import TileContext
    from concourse.tile_rust import add_dep_helper

    BF = mybir.dt.bfloat16
    F32 = mybir.dt.float32

    nc = bass.Bass("TRN2", target_bir_lowering=False, debug=False,
                   num_devices=NCORES)
    NF = 2 * D + 1    # 17 feature rows
    xs_ext = nc.dram_tensor("xs", [2 * NF, N_LOC], BF, kind="ExternalInput")
    w1_ext = nc.dram_tensor("w1", [2 * NF, NPAIR * 128], BF,
                            kind="ExternalInput")
    w2_ext = nc.dram_tensor("w2", [NF, NPAIR * 128], BF,
                            kind="ExternalInput")
    pqs_ext = nc.dram_tensor("pqs", [128, 648], BF, kind="ExternalInput")
    ov_ext = nc.dram_tensor("outv", [32, N_LOC], BF, kind="ExternalOutput")

    with TileContext(nc) as tc:
        with tc.tile_pool(name="sb", bufs=1) as sb, \
             tc.tile_pool(name="kp", bufs=8) as kp, \
             tc.tile_pool(name="gp", bufs=8) as gp, \
             tc.tile_pool(name="st", bufs=3, space="PSUM") as stp, \
             tc.tile_pool(name="pa", bufs=1, space="PSUM") as pap:
            funnel = []
            xs_d = sb.tile([2 * NF, N_LOC], BF, tag="xs_d")
            funnel.append(nc.sync.dma_start(out=xs_d[:], in_=xs_ext[:]).ins)
            w1_d = sb.tile([2 * NF, NPAIR * 128], BF, tag="w1_d")
            funnel.append(nc.sync.dma_start(out=w1_d[:], in_=w1_ext[:]).ins)
            w2_d = sb.tile([NF, NPAIR * 128], BF, tag="w2_d")
            funnel.append(nc.sync.dma_start(out=w2_d[:], in_=w2_ext[:]).ins)
            pqs_d = sb.tile([128, 648], BF, tag="pqs_d")
            funnel.append(nc.sync.dma_start(out=pqs_d[:], in_=pqs_ext[:]).ins)

            # launder DMA'd inputs (DMA-queue waits never elide; engine sems do)
            xs = sb.tile([2 * NF, N_LOC], BF, tag="xs")
            nc.scalar.copy(xs[:], xs_d[:])
            w1 = sb.tile([2 * NF, NPAIR * 128], BF, tag="w1")
            nc.scalar.copy(w1[:], w1_d[:])
            w2 = sb.tile([NF, NPAIR * 128], BF, tag="w2")
            nc.scalar.copy(w2[:], w2_d[:])
            # cr built on device from the packed pqs: Q blocks + mu-weight
            # columns land at their block-diagonal positions (aligned copies)
            cr = sb.tile([128, 1280], BF, tag="cr")
            nc.vector.memset(cr[:], 0.0)
            for ho in range(NHO):
                p, s = divmod(ho, 2)
                c0 = 128 * p + 64 * s
                nc.vector.tensor_copy(
                    cr[64 * s:64 * s + 64, c0:c0 + 64],
                    pqs_d[64 * s:64 * s + 64, 64 * p:64 * p + 64])
                mc = 1024 + 32 * p + 4 + 2 * p + s
                nc.vector.tensor_copy(
                    cr[64 * s:64 * s + 64, mc:mc + 1],
                    pqs_d[64 * s:64 * s + 64, 512 + p:513 + p])
            # mm3a one-hot pattern built on device (input-independent)
            m3aw = sb.tile([128, NPAIR * 32], BF, tag="m3aw")
            nc.vector.memset(m3aw[:], 0.0)
            for ho in range(NHO):
                p, s = divmod(ho, 2)
                w_idx = p // 2
                mc = 32 * p + 2 * (p - 2 * w_idx) + s
                nc.vector.memset(m3aw[64 * s:64 * s + 64, mc:mc + 1], 1.0)
            mmvw = sb.tile([2, 128], BF, tag="mmvw")
            nc.vector.tensor_copy(mmvw[:], pqs_d[0:2, 520:648])
            onesrow = sb.tile([2, BLK], BF, tag="onesrow")
            nc.vector.memset(onesrow[:], 1.0)
            dummy_bf = sb.tile([1, 1], BF, tag="dummy_bf")
            nc.vector.memset(dummy_bf[:], 0.0)
            dummy_srcA = sb.tile([1, 1], mybir.dt.float32, tag="dummy_srcA")
            nc.scalar.copy(dummy_srcA[:], dummy_bf[:])

            # bf16 staging: DVE converts the f32 PSUM rows on copy-out,
            # halving the result DMA/fetch bytes (tolerance has ~5x slack)
            stag_v = sb.tile([128, N_LOC], BF, tag="stag_v")

            prod_hist = []
            exp_hist = []
            mm2_hist = []
            last_pe = None
            last_dve_st = None
            last_act_st = None

            scv_prev = None
            for b in range(NBLK):
                psA = pap.tile([128, BLK], mybir.dt.float32, tag="psA")
                if scv_prev is not None:
                    ldwv = nc.tensor.ldweights(dummy_bf[:])
                    add_dep_helper(ldwv.ins, scv_prev, True,
                                   "PE observes stag_v copy before psA reuse")
                for c in range(2):
                    sl = slice(512 * c, 512 * (c + 1))
                    mmv = nc.tensor.matmul(psA[:, sl], mmvw[:],
                                           onesrow[:, sl],
                                           start=True, stop=False)
                    if scv_prev is not None:
                        add_dep_helper(mmv.ins, ldwv.ins, False, "order")
                blk_pre = []
                if b > 0:
                    prev_prod = prod_hist[b * NPAIR - 1]
                    prev_exp = exp_hist[b * NPAIR - 1]
                    t1 = sb.tile([1, 1], mybir.dt.float32, tag=f"aab1_{b}")
                    aab1 = nc.scalar.copy(t1[:], dummy_bf[:])
                    add_dep_helper(aab1.ins, prev_prod, True, "ACT sees DVE")
                    t2 = sb.tile([1, 1], mybir.dt.float32, tag=f"aab2_{b}")
                    aab2 = nc.scalar.copy(t2[:], dummy_srcA[:])
                    add_dep_helper(aab2.ins, prev_exp, True, "ACT WAW")
                    t3 = sb.tile([1, 1], mybir.dt.float32, tag=f"dvb_{b}")
                    dvb = nc.vector.memset(t3[:], 0.0)
                    add_dep_helper(dvb.ins, prev_prod, True, "DVE WAW")
                    blk_pre = [aab1.ins, aab2.ins, dvb.ins]

                for p in range(NPAIR):
                    it = b * NPAIR + p
                    w_idx = p // 2
                    ps_s = stp.tile([128, BLK], mybir.dt.float32, tag="st")
                    for c in range(2):
                        sl = slice(512 * c, 512 * (c + 1))
                        xsl = slice(BLK * b + 512 * c, BLK * b + 512 * (c + 1))
                        nc.tensor.matmul(
                            ps_s[:, sl], w1[:, 128 * p:128 * (p + 1)],
                            xs[:, xsl], start=True, stop=False)
                        nc.tensor.matmul(
                            ps_s[:, sl], w2[:, 128 * p:128 * (p + 1)],
                            xs[0:NF, xsl], start=False, stop=True)
                    kfu = kp.tile([128, BLK], BF, tag="kfu")
                    ex = nc.scalar.activation(
                        kfu[:], ps_s[:], mybir.ActivationFunctionType.Exp)
                    for pre in blk_pre:
                        add_dep_helper(ex.ins, pre, False, "after blk absorb")
                    exp_hist.append(ex.ins)
                    # absorb the ps_t slot's WAR (DVE prod of previous
                    # tenant) and PE WAW (mm1 wrote the slot this pair)
                    if it >= 1:
                        ldw = nc.tensor.ldweights(dummy_bf[:])
                        add_dep_helper(ldw.ins, prod_hist[it - 1], True,
                                       "absorb ps_t WAR")
                    ldw2 = nc.tensor.ldweights(dummy_bf[:])
                    add_dep_helper(ldw2.ins, ex.ins, True,
                                   "PE observes exp so mm2 keeps only WAW")
                    ps_t = stp.tile([128, BLK], mybir.dt.float32, tag="st")
                    mm2_first = None
                    for c in range(2):
                        sl = slice(512 * c, 512 * (c + 1))
                        mm2 = nc.tensor.matmul(ps_t[:, sl],
                                               cr[:, 128 * p:128 * (p + 1)],
                                               kfu[:, sl], start=True, stop=True)
                        if mm2_first is None:
                            mm2_first = mm2.ins
                            add_dep_helper(mm2.ins, ldw2.ins, False,
                                           "mm2 after WAW absorb")
                    mm2_hist.append(mm2.ins)
                    ddv = sb.tile([1, 1], mybir.dt.float32, tag=f"ddv{it}")
                    dab = nc.vector.memset(ddv[:], 0.0)
                    add_dep_helper(dab.ins, ex.ins, True, "absorb exp for DVE")
                    g = gp.tile([128, BLK], BF, tag="g")
                    pr = nc.vector.tensor_tensor(g[:], kfu[:], ps_t[:],
                                                 mybir.AluOpType.mult)
                    add_dep_helper(pr.ins, dab.ins, False, "order after absorb")
                    prod_hist.append(pr.ins)
                    # mm3a: bf16 window-packed var reduction
                    lc = 32 * p
                    for c in range(2):
                        sl = slice(512 * c, 512 * (c + 1))
                        nc.tensor.matmul(
                            psA[32 * w_idx:32 * w_idx + 32, sl],
                            m3aw[:, lc:lc + 32], g[:, sl],
                            start=False, stop=(p == NPAIR - 1),
                            tile_position=(0, 32 * w_idx))
                    # mm3b: f32r mu reduction at (0,0), 2 chunks
                    for c in range(2):
                        sl = slice(512 * c, 512 * (c + 1))
                        mm3b = nc.tensor.matmul(
                            psA[0:32, sl], cr[:, 1024 + 32 * p:1024 + 32 * (p + 1)],
                            kfu[:, sl], start=False, stop=False)
                        add_dep_helper(mm3b.ins, mm2_first, False,
                                       "mm3b after mm2 so ACT dep elides")
                    last_pe = mm3b.ins
                scv = nc.vector.tensor_copy(stag_v[:, BLK * b:BLK * (b + 1)],
                                            psA[:])
                scv_prev = scv.ins
                last_dve_st = scv.ins
                last_act_st = exp_hist[-1]

            # emit only the 32 live rows: var w0 + all mu, then var w1..w3.
            # 5 input DMAs keep the first tail DMA on a fresh semaphore
            # slot, so it carries only the staging-DVE wait (1-wait limit).
            funnel.append(nc.sync.dma_start(out=ov_ext[0:20, :],
                                            in_=stag_v[0:20, :]).ins)
            funnel.append(nc.sync.dma_start(out=ov_ext[20:24, :],
                                            in_=stag_v[32:36, :]).ins)
            funnel.append(nc.sync.dma_start(out=ov_ext[24:28, :],
                                            in_=stag_v[64:68, :]).ins)
            funnel.append(nc.sync.dma_start(out=ov_ext[28:32, :],
                                            in_=stag_v[96:100, :]).ins)
            funnel += [last_pe, last_dve_st, last_act_st, prod_hist[-1]]
            for dep in funnel:
                nop = nc.sync.nop(nofuse=True)
                add_dep_helper(nop.ins, dep, True, "tail funnel")
    return nc


def _build_runner():
    """Build the Bass program and a cached shard_map jit around bass_exec."""
    import jax
    from jax.sharding import Mesh, PartitionSpec
    from jax.experimental.shard_map import shard_map
    import concourse.mybir as mybir
    from concourse.bass2jax import (_bass_exec_p, partition_id_tensor,
                                    install_neuronx_cc_hook)

    nc = _build_program()
    install_neuronx_cc_hook()

    partition_name = (nc.partition_id_tensor.name
                      if nc.partition_id_tensor else None)
    in_names, out_names, out_avals = [], [], []
    for alloc in nc.m.functions[0].allocations:
        if not isinstance(alloc, mybir.MemoryLocationSet):
            continue
        name = alloc.memorylocations[0].name
        if alloc.kind == "ExternalInput":
            if name != partition_name:
                in_names.append(name)
        elif alloc.kind == "ExternalOutput":
            out_names.append(name)
            out_avals.append(jax.core.ShapedArray(
                tuple(alloc.tensor_shape), mybir.dt.np(alloc.dtype)))
    n_params = len(in_names)
    all_names = list(in_names) + list(out_names)
    if partition_name is not None:
        all_names.append(partition_name)

    def _body(*args):
        operands = list(args)
        if partition_name is not None:
            operands.append(partition_id_tensor())
        outs = _bass_exec_p.bind(
            *operands,
            out_avals=tuple(out_avals),
            in_names=tuple(all_names),
            out_names=tuple(out_names),
            lowering_input_output_aliases=(),
            sim_require_finite=True,
            sim_require_nnan=True,
            nc=nc,
        )
        return tuple(outs)

    devices = jax.devices()[:NCORES]
    mesh = Mesh(np.asarray(devices), ("core",))
    donate = tuple(range(n_params, n_params + len(out_names)))
    sharded = jax.jit(
        shard_map(_body, mesh=mesh,
                  in_specs=(PartitionSpec("core"),) * (n_params + len(out_names)),
                  out_specs=(PartitionSpec("core"),) * len(out_names),
                  check_rep=False),
        donate_argnums=donate, keep_unused=True)
    _cache["nc"] = nc
    _cache["sharded"] = sharded
    _cache["in_names"] = in_names
    # device-resident donor so every call has the same arg signature
    # (numpy zeros on call 1 vs donated jax.Array later would retrace)
    from jax.sharding import NamedSharding
    _cache["donor"] = jax.device_put(
        np.zeros((NCORES * 32, N_LOC), BF16),
        NamedSharding(mesh, PartitionSpec("core")))
    _cache["pool"] = ThreadPoolExecutor(NCORES)


def kernel(x, z, u_mean, u_tril_vec, log_ls, log_var):
    if "sharded" not in _cache:
        _build_runner()

    xs, w1, w2, pqs = _host_precompute(
        np.asarray(x), np.asarray(z), np.asarray(u_mean),
        np.asarray(u_tril_vec), np.asarray(log_ls), np.asarray(log_var))

    globals_by_name = {
        "xs": xs.reshape(34, NCORES, N_LOC).transpose(1, 0, 2)
                .reshape(NCORES * 34, N_LOC),
        "w1": np.tile(w1, (NCORES, 1)),
        "w2": np.tile(w2, (NCORES, 1)),
        "pqs": np.tile(pqs, (NCORES, 1)),
    }
    args = [globals_by_name[n] for n in _cache["in_names"]]
    args.append(_cache["donor"])
    out = _cache["sharded"](*args)[0]
    _cache["donor"] = out

    shards = sorted(out.addressable_shards, key=lambda s: s.index[0].start)
    parts = list(_cache["pool"].map(lambda s: np.asarray(s.data), shards))
    full = np.concatenate(parts, axis=1).astype(np.float32)   # [32, N]

    pred_mu = np.empty((NHO, N), np.float32)
    pred_var = np.empty((NHO, N), np.float32)
    for ho in range(NHO):
        p, s = divmod(ho, 2)
        w_idx = p // 2
        pred_var[ho] = full[VAR_BASE[w_idx] + 2 * (p - 2 * w_idx) + s]
        pred_mu[ho] = full[4 + 2 * p + s]
    return (pred_mu.reshape(H, O, N), pred_var.reshape(H, O, N))
